# revision 13
# baseline (speedup 1.0000x reference)
"""Trainium2 Bass kernel for nn_Nested_Res2Net_TDNN (B=32, CIN=1024, T=600).

Sharding: data-parallel over batch across 8 NeuronCores (4 per core),
parameters replicated.

v2: fp8(e4m3) DoubleRow matmuls.  Data stays channels-on-128-partitions,
(batch x time + guard pads) on the free dim, exactly as the fp16 version;
every conv becomes fp8 DoubleRow pairs at 0.5 cycles/column:
  - the 3 dilated taps of each scale branch pair as (tap-2, tap0) via a
    stride-2 rhs access pattern, plus (tap+2, fp8-error-compensation) via
    a stride-0 pair,
  - the 1x1 convs (conv1 spx/OB parts, conv3 terms) pair each fp8 weight
    plane with its own fp8 quantization-error plane (stride-0 rhs), which
    recovers most of the weight-quantization error for free,
  - BN affines fold into following-matmul weights (rows scaled by k) with
    -c/k guard-pad values so dilated taps see exact zero padding,
  - post-matmul relu/bias writes are spread over ACT + DVE + GPSIMD,
  - the SE/tail is fused with scalar_tensor_tensor: u=(z3*k3g)+res in
    place, OB=relu(u+gate*c3+co_prev), pooling rides as
    (OB*kpf) max (-cpf) with accum_out, residual = (OB*ko)+spx_next.
fp16 is kept for the SE squeeze path, O3/residual tensors and the
classifier; PSUM is fp32 throughout.
"""

import sys

for _p in ("/opt/trn_rl_repo",):
    if _p not in sys.path:
        sys.path.insert(0, _p)

import numpy as np
import ml_dtypes

import concourse.bass as bass
import concourse.mybir as mybir
import concourse.tile as tile
from concourse.bass_utils import run_bass_kernel_spmd
from bass_rust import AP as RAP

B, CIN, T0 = 32, 1024, 600
NES, SCALE = 8, 8
CBLK, WSC = 128, 16
NIN, NOUT = 7, 7
EPS = 1e-5
NCORES = 8
BL = B // NCORES

F32 = mybir.dt.float32
F16 = mybir.dt.float16
F8 = mybir.dt.float8e4
E4 = ml_dtypes.float8_e4m3
ALU = mybir.AluOpType
ACTF = mybir.ActivationFunctionType
DR = mybir.MatmulPerfMode.DoubleRow

# DoubleRow weight slots (each [K=128, 2 planes, 128 out]) per block
SC1A = 0                      # conv1 spx part: (W1p, err)
SC1B = 1                      # conv1 OB part: (W1B, err)
STAPA = lambda j: 2 + 2 * j   # branch j taps (-2, 0)
STAPB = lambda j: 3 + 2 * j   # branch j (tap +2, err)
SC3P = lambda m: 16 + m       # conv3 slab pairs: (IX7,c0),(c1,c2),(c3,c4),(c5,c6)
NDR = 20

# VEC columns (fp32 per-partition scalars)
VB1, VPAD1 = 0, 1
VBRB = lambda j: 2 + 2 * j    # branch post bias
VBRP = lambda j: 3 + 2 * j    # branch pad value -c/k
VB3, VKO, VK3, VC3, VCOP = 16, 17, 18, 19, 20
VKPF, VCPF, VSE1B, VSE2B, VNCPF = 21, 22, 23, 24, 25
NV = 26


def _perm():
    """ZS slice order: s=0 is spx[7] (channels 112:128), s>=1 is spx[s-1]."""
    p = np.zeros(128, np.int64)
    for s in range(8):
        for c in range(16):
            p[16 * s + c] = 112 + c if s == 0 else 16 * (s - 1) + c
    return p


def _bnkc(p):
    g, b, m, v = [np.asarray(a, np.float64) for a in p]
    k = g / np.sqrt(v + EPS)
    return k, b - m * k


def _q8(a):
    return np.asarray(a, np.float32).astype(E4).astype(np.float64)


def _prep(inp, T):
    f = lambda n: np.asarray(inp[n], np.float64)
    w1, b1, bn1 = f("w1"), f("b1"), f("bn1")
    cw, cb, ibn = f("cw"), f("cb"), f("ibn")
    w3, b3, bn3 = f("w3"), f("b3"), f("bn3")
    se1w, se1b = f("se1w"), f("se1b")
    se2w, se2b = f("se2w"), f("se2b")
    obn, fbn = f("obn"), f("fbn")
    fcw, fcb = f("fcw"), f("fcb")
    ws = [f(f"ws{j}") for j in range(NIN)]
    perm = _perm()
    kf, cf = _bnkc(fbn)  # (1024,)

    LW8 = np.zeros((NOUT, 128, NDR, 2, 128), np.float64)
    LWF = np.zeros((NOUT, 128, 2, 128), np.float64)
    VEC = np.zeros((NOUT, 128, NV), np.float64)
    TCPF = np.zeros((128, NES * BL), np.float64)

    def put_pair(i, slot, Wm):
        """plane0 = fp8(W), plane1 = fp8(W - plane0)."""
        p0 = _q8(Wm)
        LW8[i, :, slot, 0] = p0
        LW8[i, :, slot, 1] = _q8(Wm - p0)

    for i in range(NOUT):
        k1, c1 = _bnkc(bn1[i])
        k1p, c1p = k1[perm], c1[perm]
        kb, cbv = [], []
        for j in range(NIN):
            kj, cj = _bnkc(ibn[i, j])
            kb.append(kj)
            cbv.append(cj)
        assert np.abs(k1).min() > 1e-3
        assert min(np.abs(k).min() for k in kb) > 1e-3

        def row_kc(j, k1p=k1p, c1p=c1p, kb=kb, cbv=cbv):
            k = k1p.copy()
            c = c1p.copy()
            if j > 0:
                n = 16 * (j + 1)
                k[:n] = np.tile(kb[j - 1], j + 1)
                c[:n] = np.tile(cbv[j - 1], j + 1)
            return k, c

        put_pair(i, SC1A, w1[i][perm, :].T)
        ko, co = _bnkc(obn[i])
        if i > 0:
            # OB is stored pre-scaled by ko_prev, so no fold here
            put_pair(i, SC1B, w1[i][perm, :].T)

        for j in range(NIN):
            krow, crow = row_kc(j)
            blk = [cw[i, j, :, :, d, 0].T for d in range(3)]  # [ci, co]
            tapm = [np.zeros((128, 128)) for _ in range(3)]
            bias_full = np.zeros(128)
            for s in range(j + 2):
                r0 = 16 * s
                const_s = np.zeros(16)
                for d in range(3):
                    tapm[d][r0:r0 + 16, r0:r0 + 16] = \
                        blk[d] * krow[r0:r0 + 16, None]
                    const_s += blk[d].T @ crow[r0:r0 + 16]
                bias_full[r0:r0 + 16] = cb[i, j] + const_s
            # slot A: (tap-2, tap0); slot B: (tap+2, err(tap+2))
            LW8[i, :, STAPA(j), 0] = _q8(tapm[0])
            LW8[i, :, STAPA(j), 1] = _q8(tapm[1])
            put_pair(i, STAPB(j), tapm[2])
            VEC[i, :, VBRB(j)] = bias_full
            VEC[i, :16 * (j + 2), VBRP(j)] = \
                np.tile(-cbv[j] / kb[j], j + 2)

        # conv3 terms, packed as slab pairs (no err compensation)
        c3const = np.zeros(128)
        IX7 = np.zeros((128, 128))
        IX7[0:16, :] = w3[i][:, 112:128].T * k1p[0:16, None]
        c3const += w3[i][:, 112:128] @ c1p[0:16]
        IC3s = []
        for j in range(NIN):
            wj = ws[j][i]
            IC3 = np.zeros((128, 128))
            for s in range(j + 2):
                r0 = 16 * s
                IC3[r0:r0 + 16, :] = \
                    (wj[s] * w3[i][:, 16 * j:16 * j + 16].T) * kb[j][:, None]
                c3const += wj[s] * (w3[i][:, 16 * j:16 * j + 16] @ cbv[j])
            IC3s.append(IC3)
        # pair m reads (slab0-term, slab1-term) at the post(2m) epoch
        pairs3 = [(IX7, IC3s[0]), (IC3s[1], IC3s[2]),
                  (IC3s[3], IC3s[4]), (IC3s[5], IC3s[6])]
        for m, (pa, pb) in enumerate(pairs3):
            LW8[i, :, SC3P(m), 0] = _q8(pa)
            LW8[i, :, SC3P(m), 1] = _q8(pb)

        k3, c3 = _bnkc(bn3[i])
        LWF[i, :, 0, 0:16] = (se1w[i] * k3[None, :] / T).T
        LWF[i, 0:16, 1, :] = se2w[i].T

        VEC[i, :, VB1] = b1[i][perm]
        if i > 0:
            _, co_p = _bnkc(obn[i - 1])
            VEC[i, :, VB1] += (w1[i] @ co_p)[perm]
            VEC[i, :, VCOP] = ko * co_p   # obias' carries the ko prescale
        VEC[i, :, VPAD1] = -c1p / k1p
        VEC[i, :, VB3] = b3[i] + c3const
        VEC[i, :, VKO] = ko
        VEC[i, :, VK3], VEC[i, :, VC3] = k3, ko * c3
        kfi = kf[128 * i:128 * (i + 1)]
        cfi = cf[128 * i:128 * (i + 1)]
        VEC[i, :, VKPF] = kfi           # OB' = ko*OB already
        VEC[i, :, VCPF] = kfi * co + cfi
        VEC[i, :, VNCPF] = -(kfi * co + cfi)
        TCPF[:, BL * i:BL * (i + 1)] = (T * (kfi * co + cfi))[:, None]
        VEC[i, :16, VSE1B] = se1b[i] + se1w[i] @ c3
        VEC[i, :, VSE2B] = se2b[i]
    TCPF[:, 30:32] = (T * cf[896:1024])[:, None]  # x7 b2,b3 use max-form
    vec7 = np.stack([kf[896:1024], cf[896:1024], -cf[896:1024]], axis=1)
    fcwp = np.zeros((128, 8, 2))
    for g in range(8):
        fcwp[:, g, :] = (fcw[:, 128 * g:128 * (g + 1)] / T).T
    lw8 = np.ascontiguousarray(
        LW8.reshape(NOUT, 128, NDR * 256).astype(E4))
    lwf = np.ascontiguousarray(
        LWF.reshape(NOUT, 128, 256).astype(np.float16))
    vec = np.ascontiguousarray(VEC.astype(np.float32))
    return (lw8, lwf, vec,
            np.ascontiguousarray(TCPF.astype(np.float32)),
            np.ascontiguousarray(vec7.astype(np.float32)),
            np.ascontiguousarray(fcwp.reshape(128, 16).astype(np.float16)),
            np.ascontiguousarray(fcb.reshape(2, 1).astype(np.float32)))


def _split_waits(nc, max_waits=1):
    """walrus's TRN2 codegen rejects >1 sync wait on drain/matmul (and
    possibly other) instructions; peel extras onto preceding single-wait
    no-ops on the same engine."""
    n_new = 0
    for fn in nc.m.functions:
        for bb in fn.blocks:
            out = []
            for ins in bb.instructions:
                si = ins.sync_info
                if si is not None and len(si.on_wait) > max_waits:
                    waits = list(si.on_wait)
                    for w in waits[max_waits:]:
                        nop = mybir.InstNoOp(
                            name=f"I-splitwait-{n_new}",
                            sync_info=mybir.SyncInfo(on_wait=[w], on_update=[]),
                            bass_nofuse=True,
                            engine=ins.engine,
                        )
                        out.append(nop)
                        n_new += 1
                    ins.sync_info = mybir.SyncInfo(
                        on_wait=waits[:max_waits], on_update=list(si.on_update))
                out.append(ins)
            bb.instructions = out
    return n_new


def build(T=T0):
    nc = bass.Bass("TRN2")
    P = T + 4
    W2 = BL * P + 4
    lo, hi = 2, 2 + BL * P
    chunks = []
    c = lo
    while c < hi:
        w = min(512, hi - c)
        chunks.append((c, w))
        c += w
    NCH = len(chunks)
    # valid-column pieces (for bn3 squeeze accumulation)
    pieces = []
    for b in range(BL):
        v0, v1 = 4 + P * b, 4 + P * b + T
        for ci, (c0, w) in enumerate(chunks):
            s, e = max(v0, c0), min(v1, c0 + w)
            if s < e:
                pieces.append((ci, s, e - s, b))
    NP = len(pieces)
    assert NP == 2 * BL and all(p[3] == k // 2 for k, p in enumerate(pieces))

    x_d = nc.dram_tensor("x", [BL, CIN, T], F8, kind="ExternalInput")
    xr_d = nc.dram_tensor("xr", [BL, 128, T], F16, kind="ExternalInput")
    lw8_d = nc.dram_tensor("lw8", [NOUT, 128, NDR * 256], F8,
                           kind="ExternalInput")
    lwf_d = nc.dram_tensor("lwf", [NOUT, 128, 256], F16, kind="ExternalInput")
    vec_d = nc.dram_tensor("vec", [NOUT, 128, NV], F32, kind="ExternalInput")
    tcpf_d = nc.dram_tensor("tcpf", [128, NES * BL], F32, kind="ExternalInput")
    vec7_d = nc.dram_tensor("vec7", [128, 3], F32, kind="ExternalInput")
    fcw_d = nc.dram_tensor("fcw", [128, 16], F16, kind="ExternalInput")
    fcb_d = nc.dram_tensor("fcb", [2, 1], F32, kind="ExternalInput")
    y_d = nc.dram_tensor("y", [2, BL], F32, kind="ExternalOutput")

    def pads_ap(t, nact, slab=None):
        """All guard/pad columns: 4-col blocks at 0, P, 2P, 3P, 4P."""
        a = t[:]
        off = a.offset + (0 if slab is None else slab * W2)
        return RAP(a.tensor, off,
                   [[int(a.ap[0][0]), nact], [P, BL + 1], [1, 4]])

    def pair(t, base, stride, w, np_=128, slab=None):
        a = t[:]
        off = a.offset + base + (0 if slab is None else slab * W2)
        return RAP(a.tensor, off,
                   [[int(a.ap[0][0]), np_], [stride, 2], [1, w]])

    with tile.TileContext(nc) as tc:
        with tc.tile_pool(name="state", bufs=1) as state, \
             tc.tile_pool(name="spxp", bufs=2) as spxp, \
             tc.tile_pool(name="wp", bufs=2) as wp, \
             tc.tile_pool(name="vp", bufs=2) as vp, \
             tc.tile_pool(name="bpsum", bufs=2, space="PSUM") as bpsum, \
             tc.tile_pool(name="cpsum", bufs=NCH, space="PSUM") as cpsum, \
             tc.tile_pool(name="spsum", bufs=1, space="PSUM") as spsum:

            ZS2 = state.tile([128, 2, W2], F8, tag="ZS2")
            OB = state.tile([128, W2], F8, tag="OB")
            Rb = state.tile([128, W2], F16, tag="Rb")
            O3 = state.tile([128, W2], F16, tag="O3")
            scrp = state.tile([128, BL, T], F16, tag="scrp")
            sq8 = state.tile([128, NP], F32, tag="sq8")
            sq_r = state.tile([128, BL], F16, tag="sq_r")
            seh = state.tile([128, BL], F16, tag="seh")
            gate = state.tile([128, BL], F32, tag="gate")
            k3g = state.tile([128, BL], F32, tag="k3g")
            obias = state.tile([128, BL], F32, tag="obias")
            Mt = state.tile([128, NES * BL], F32, tag="Mt")
            Mt_r = state.tile([128, NES * BL], F16, tag="Mt_r")
            zero16 = state.tile([128, 1], F16, tag="z16")
            outs = state.tile([2, BL], F32, tag="outs")
            fcw_s = state.tile([128, 16], F16, tag="fcw")
            fcb_s = state.tile([2, 1], F32, tag="fcb")
            tcpf_s = state.tile([128, NES * BL], F32, tag="tcpf")
            vec7_s = state.tile([128, 3], F32, tag="vec7")

            # conv3 slab pairs read all 128 slab1 rows (zero weights on
            # unused rows) before the first block fully populates them
            nc.gpsimd.memset(ZS2[32:64, 1, :], 0)
            nc.gpsimd.memset(ZS2[64:128, 1, :], 0)
            nc.gpsimd.memset(OB[:], 0)
            nc.vector.memset(zero16[:], 0)
            nc.sync.dma_start(out=fcw_s[:], in_=fcw_d[:])
            nc.sync.dma_start(out=fcb_s[:], in_=fcb_d[:])
            nc.sync.dma_start(out=vec7_s[:], in_=vec7_d[:])
            nc.sync.dma_start(out=tcpf_s[:], in_=tcpf_d[:])
            # residual for block 0 = x[:, 0:128] in fp16
            nc.sync.dma_start(
                out=Rb[:, lo:hi].rearrange("p (b q) -> p b q", q=P)[:, :, 2:T + 2],
                in_=xr_d.rearrange("b c t -> c b t"))

            def load_spx(i):
                t = spxp.tile([128, W2], F8, tag="spx")
                nc.gpsimd.memset(pads_ap(t, 128), 0)
                nc.sync.dma_start(
                    out=t[:, lo:hi].rearrange("p (b q) -> p b q", q=P)[:, :, 2:T + 2],
                    in_=x_d[:, 128 * i:128 * (i + 1), :].rearrange("b c t -> c b t"))
                return t

            def load_w(i):
                t = wp.tile([128, NDR, 256], F8, tag="lw8")
                nc.sync.dma_start(
                    out=t[:], in_=lw8_d[i].rearrange("k (s m) -> k s m", m=256))
                tf = wp.tile([128, 2, 128], F16, tag="lwf")
                nc.sync.dma_start(
                    out=tf[:], in_=lwf_d[i].rearrange("k (s m) -> k s m", m=128))
                v = vp.tile([128, NV], F32, tag="vec")
                nc.sync.dma_start(out=v[:], in_=vec_d[i])
                return t, tf, v

            # post-op engine rotation.  GPSIMD cannot read PSUM on HW, so
            # PSUM drains go on ACT/DVE only (ACT is faster per column).
            _rr = [0]
            _pat = "aadadadadadad"  # 8 ACT : 5 DVE

            def post_chunk(dst, ps, nact, biasap, accum=None):
                e = _pat[_rr[0] % len(_pat)]
                _rr[0] += 1
                if e == "a":
                    nc.scalar.activation(dst, ps, ACTF.Relu, bias=biasap,
                                         scale=1.0, accum_out=accum)
                else:
                    if accum is None:
                        nc.vector.tensor_scalar(dst, ps, biasap, 0.0,
                                                ALU.add, ALU.max)
                    else:
                        nc.vector.scalar_tensor_tensor(
                            dst, ps, biasap,
                            zero16[:nact, 0:1].to_broadcast(list(dst.shape)),
                            ALU.add, ALU.max, accum_out=accum)

            def pad_blk(t, nact, Vt, col, blkidx, slab):
                a = t[:]
                off = a.offset + slab * W2 + P * blkidx
                ap = RAP(a.tensor, off, [[int(a.ap[0][0]), nact], [1, 4]])
                bc = Vt[:nact, col:col + 1].to_broadcast([nact, 4])
                nc.gpsimd.tensor_copy(out=ap, in_=bc)

            def post_pass(dst, psl, nact, bias_col, Vt, padt=None,
                          pad_col=None, pad_slab=0):
                biasap = Vt[:nact, bias_col:bias_col + 1]
                for k, (c0, w) in enumerate(chunks):
                    post_chunk(dst[:nact, c0:c0 + w], psl[k][:nact, :w],
                               nact, biasap)
                    if padt is not None:
                        pad_blk(padt, nact, Vt, pad_col, k, pad_slab)

            def w8ap(Wt, slot, nact=128):
                return Wt[:, slot, :].rearrange("k (a m) -> k a m", a=2)[:, :, 0:nact]

            spx = load_spx(0)
            LWt, LWf, Vt = load_w(0)
            pend_c1 = None

            for i in range(NOUT):
                nxt = load_spx(i + 1)
                nW = load_w(i + 1) if i < NOUT - 1 else None

                # conv1 psum: spx part was issued in block i-1's tail
                if i == 0:
                    c1ps = [cpsum.tile([128, 512], F32, tag="cps",
                                       name="c1ps")[:, :w] for (c0, w) in chunks]
                    for k, (c0, w) in enumerate(chunks):
                        nc.tensor.matmul(c1ps[k], w8ap(LWt, SC1A),
                                         pair(spx, c0, 0, w),
                                         start=True, stop=True, perf_mode=DR)
                else:
                    c1ps = pend_c1  # completed (SC1B) in prev block's tail
                zs0 = ZS2[:, 0, :]
                post_pass(zs0, c1ps, 128, VB1, Vt,
                          padt=ZS2, pad_col=VPAD1, pad_slab=0)
                # pristine rows needed by odd branches on slab 1 (incl pads)
                for r0 in (32, 64, 96):
                    nc.sync.dma_start(out=ZS2[r0:r0 + 16, 1, :],
                                      in_=ZS2[r0:r0 + 16, 0, :])

                # conv3 accumulation group (filled by slab pairs below)
                cps = [cpsum.tile([128, 512], F32, tag="cps", name="cps")[:, :w]
                       for (c0, w) in chunks]

                # inner scale branches, in place in ZS
                for j in range(NIN):
                    nact = 16 * (j + 2)
                    rj, wj = j % 2, (j + 1) % 2
                    bps = [bpsum.tile([128, 512], F32, tag="bps",
                                      name="bps")[:, :w] for (c0, w) in chunks]
                    # chunk-pair interleave: with bufs=2 PSUM banks, group k
                    # must close (TAPB) before bank reuse at chunk k+2
                    for kk in range(0, NCH, 2):
                        ks = [k for k in (kk, kk + 1) if k < NCH]
                        for k in ks:
                            c0, w = chunks[k]
                            nc.tensor.matmul(bps[k][:nact, :],
                                             w8ap(LWt, STAPA(j), nact),
                                             pair(ZS2, c0 - 2, 2, w, slab=rj),
                                             start=True, stop=False,
                                             perf_mode=DR)
                        for k in ks:
                            c0, w = chunks[k]
                            nc.tensor.matmul(bps[k][:nact, :],
                                             w8ap(LWt, STAPB(j), nact),
                                             pair(ZS2, c0 + 2, 0, w, slab=rj),
                                             start=False, stop=True,
                                             perf_mode=DR)
                    post_pass(ZS2[:, wj, :], bps, nact, VBRB(j), Vt,
                              padt=ZS2, pad_col=VBRP(j), pad_slab=wj)
                    if j % 2 == 0:
                        # conv3 slab pair m=j//2: (slab0 term, slab1 term)
                        for k, (c0, w) in enumerate(chunks):
                            nc.tensor.matmul(cps[k], w8ap(LWt, SC3P(j // 2)),
                                             pair(ZS2, c0, W2, w),
                                             start=(j == 0), stop=(j == 6),
                                             perf_mode=DR)

                # bn3 relu -> O3 (fp16), per-piece accum for SE squeeze
                for kpc, (ci, s, w, b) in enumerate(pieces):
                    c0 = chunks[ci][0]
                    post_chunk(O3[:, s:s + w], cps[ci][:, s - c0:s - c0 + w],
                               128, Vt[:, VB3:VB3 + 1],
                               accum=sq8[:, kpc:kpc + 1])
                # next block's conv1 spx-part (fills the SE gap)
                if nW is not None:
                    pend_c1 = [cpsum.tile([128, 512], F32, tag="cps",
                                          name="c1ps")[:, :w]
                               for (c0, w) in chunks]
                    for k, (c0, w) in enumerate(chunks):
                        nc.tensor.matmul(pend_c1[k], w8ap(nW[0], SC1A),
                                         pair(nxt, c0, 0, w),
                                         start=True, stop=False, perf_mode=DR)

                # SE squeeze/excite (fp16)
                nc.vector.tensor_tensor(
                    sq_r[:], sq8[:, 0:NP:2], sq8[:, 1:NP:2], ALU.add)
                ps1 = spsum.tile([128, BL], F32, tag="sps")
                nc.tensor.matmul(ps1, LWf[:, 0, :], sq_r[:],
                                 start=True, stop=True)
                nc.scalar.activation(seh[:], ps1, ACTF.Relu,
                                     bias=Vt[:, VSE1B:VSE1B + 1], scale=1.0)
                ps2 = spsum.tile([128, BL], F32, tag="sps")
                nc.tensor.matmul(ps2, LWf[:, 1, :], seh[:],
                                 start=True, stop=True)
                nc.scalar.activation(gate[:], ps2, ACTF.Sigmoid,
                                     bias=Vt[:, VSE2B:VSE2B + 1], scale=1.0)
                # gate-scaled scalars: k3g = gate*k3, obias = gate*c3 + co_prev
                nc.vector.tensor_tensor(
                    k3g[:], gate[:],
                    Vt[:, VK3:VK3 + 1].to_broadcast([128, BL]), ALU.mult)
                nc.vector.scalar_tensor_tensor(
                    obias[:], gate[:], Vt[:, VC3:VC3 + 1],
                    Vt[:, VCOP:VCOP + 1].to_broadcast([128, BL]),
                    ALU.mult, ALU.add)

                # tail per batch: u = O3*k3g + res (in place);
                # OB = relu(u + obias); pool accum; Rb = OB*ko + nxt
                c1b_chunks = {0: [0], 1: [1], 2: [2], 3: [3, 4]}
                for b in range(BL):
                    v0 = 4 + P * b
                    o3b = O3[:, v0:v0 + T]
                    nc.vector.scalar_tensor_tensor(
                        o3b, o3b, k3g[:, b:b + 1], Rb[:, v0:v0 + T],
                        ALU.mult, ALU.add)
                    obb = OB[:, v0:v0 + T]
                    nc.scalar.activation(obb, o3b, ACTF.Relu,
                                         bias=obias[:, b:b + 1],
                                         scale=Vt[:, VKO:VKO + 1])
                    # complete next block's conv1 psum group on the chunks
                    # this batch unblocks (PE gets work while tail runs)
                    if nW is not None:
                        for k in c1b_chunks[b]:
                            c0, w = chunks[k]
                            nc.tensor.matmul(pend_c1[k], w8ap(nW[0], SC1B),
                                             pair(OB, c0, 0, w),
                                             start=False, stop=True,
                                             perf_mode=DR)
                # pooling on DVE (max-form; +T*cpf re-added via TCPF map)
                for b in range(BL):
                    v0 = 4 + P * b
                    col = BL * i + b
                    nc.vector.scalar_tensor_tensor(
                        scrp[:, b, :], OB[:, v0:v0 + T],
                        Vt[:, VKPF:VKPF + 1],
                        Vt[:, VNCPF:VNCPF + 1].to_broadcast([128, T]),
                        ALU.mult, ALU.max, accum_out=Mt[:, col:col + 1])
                if i < NOUT - 1:
                    rbv = Rb[:, lo:hi].rearrange("p (b q) -> p b q", q=P)
                    obv = OB[:, lo:hi].rearrange("p (b q) -> p b q", q=P)
                    nxv = nxt[:, lo:hi].rearrange("p (b q) -> p b q", q=P)
                    nc.gpsimd.tensor_tensor(
                        rbv[:, :, 2:T + 2], obv[:, :, 2:T + 2],
                        nxv[:, :, 2:T + 2], ALU.add)
                spx = nxt
                if nW is not None:
                    LWt, LWf, Vt = nW

            # final-pool contribution of raw spx[7]: relu(kf*x + cf)
            sxv = spx[:, lo:hi].rearrange("p (b q) -> p b q", q=P)
            for b in range(BL):
                col = BL * 7 + b
                if b < 2:
                    nc.scalar.activation(scrp[:, b, :], sxv[:, b, 2:T + 2],
                                         ACTF.Relu, bias=vec7_s[:, 1:2],
                                         scale=vec7_s[:, 0:1],
                                         accum_out=Mt[:, col:col + 1])
                else:
                    nc.vector.scalar_tensor_tensor(
                        scrp[:, b, :], sxv[:, b, 2:T + 2], vec7_s[:, 0:1],
                        vec7_s[:, 2:3].to_broadcast([128, T]),
                        ALU.mult, ALU.max, accum_out=Mt[:, col:col + 1])

            # classifier: y = sum_g fcw_g.T @ M_g + fcb
            nc.vector.tensor_tensor(Mt_r[:], Mt[:], tcpf_s[:], ALU.add)
            fps = spsum.tile([2, BL], F32, tag="sps", name="fps")
            for g in range(8):
                nc.tensor.matmul(fps, fcw_s[:, 2 * g:2 * g + 2],
                                 Mt_r[:, BL * g:BL * (g + 1)],
                                 start=(g == 0), stop=(g == 7))
            nc.scalar.activation(outs[:], fps, ACTF.Identity,
                                 bias=fcb_s[:], scale=1.0)
            nc.sync.dma_start(out=y_d[:], in_=outs[:])

    return nc


_NC_CACHE = {}


def _get_nc(T):
    if T not in _NC_CACHE:
        nc = build(T)
        _split_waits(nc)
        _NC_CACHE[T] = nc
    return _NC_CACHE[T]


def make_in_maps(inputs):
    x = np.asarray(inputs["x"], np.float32)
    T = x.shape[2]
    lw8, lwf, vec, tcpf, vec7, fcw, fcb = _prep(inputs, T)
    x8 = np.ascontiguousarray(x.astype(E4))
    xr = np.ascontiguousarray(x[:, 0:128, :].astype(np.float16))
    in_maps = []
    for core in range(NCORES):
        in_maps.append({
            "x": np.ascontiguousarray(x8[core * BL:(core + 1) * BL]),
            "xr": np.ascontiguousarray(xr[core * BL:(core + 1) * BL]),
            "lw8": lw8, "lwf": lwf, "vec": vec, "tcpf": tcpf,
            "vec7": vec7, "fcw": fcw, "fcb": fcb,
        })
    return in_maps, T


def kernel(**inputs):
    in_maps, T = make_in_maps(inputs)
    nc = _get_nc(T)
    res = run_bass_kernel_spmd(nc, in_maps, list(range(NCORES)))
    out = np.concatenate(
        [np.asarray(res.results[c]["y"]).T for c in range(NCORES)], axis=0)
    return np.ascontiguousarray(out.astype(np.float32))


# revision 15
# speedup vs baseline: 1.2365x; 1.2365x over previous
"""Trainium2 Bass kernel for nn_Nested_Res2Net_TDNN (B=32, CIN=1024, T=600).

Sharding: data-parallel over batch across 8 NeuronCores (4 per core),
parameters replicated.

v2: fp8(e4m3) DoubleRow matmuls.  Data stays channels-on-128-partitions,
(batch x time + guard pads) on the free dim, exactly as the fp16 version;
every conv becomes fp8 DoubleRow pairs at 0.5 cycles/column:
  - the 3 dilated taps of each scale branch pair as (tap-2, tap0) via a
    stride-2 rhs access pattern, plus (tap+2, fp8-error-compensation) via
    a stride-0 pair,
  - the 1x1 convs (conv1 spx/OB parts, conv3 terms) pair each fp8 weight
    plane with its own fp8 quantization-error plane (stride-0 rhs), which
    recovers most of the weight-quantization error for free,
  - BN affines fold into following-matmul weights (rows scaled by k) with
    -c/k guard-pad values so dilated taps see exact zero padding,
  - post-matmul relu/bias writes are spread over ACT + DVE + GPSIMD,
  - the SE/tail is fused with scalar_tensor_tensor: u=(z3*k3g)+res in
    place, OB=relu(u+gate*c3+co_prev), pooling rides as
    (OB*kpf) max (-cpf) with accum_out, residual = (OB*ko)+spx_next.
fp16 is kept for the SE squeeze path, O3/residual tensors and the
classifier; PSUM is fp32 throughout.
"""

import sys

for _p in ("/opt/trn_rl_repo",):
    if _p not in sys.path:
        sys.path.insert(0, _p)

import numpy as np
import ml_dtypes

import concourse.bass as bass
import concourse.mybir as mybir
import concourse.tile as tile
from concourse.bass_utils import run_bass_kernel_spmd
from bass_rust import AP as RAP

B, CIN, T0 = 32, 1024, 600
NES, SCALE = 8, 8
CBLK, WSC = 128, 16
NIN, NOUT = 7, 7
EPS = 1e-5
NCORES = 8
BL = B // NCORES

F32 = mybir.dt.float32
F16 = mybir.dt.float16
F8 = mybir.dt.float8e4
E4 = ml_dtypes.float8_e4m3
ALU = mybir.AluOpType
ACTF = mybir.ActivationFunctionType
DR = mybir.MatmulPerfMode.DoubleRow

# DoubleRow weight slots (each [K=128, 2 planes, 128 out]) per block
SC1A = 0                      # conv1 spx part: (W1p, err)
SC1B = 1                      # conv1 OB part: (W1B, err)
STAPA = lambda j: 2 + 2 * j   # branch j taps (-2, 0)
STAPB = lambda j: 3 + 2 * j   # branch j (tap +2, err)
SC3P = lambda m: 16 + m       # conv3 slab pairs: (IX7,c0),(c1,c2),(c3,c4),(c5,c6)
NDR = 20

# VEC columns (fp32 per-partition scalars)
VB1, VPAD1 = 0, 1
VBRB = lambda j: 2 + 2 * j    # branch post bias
VBRP = lambda j: 3 + 2 * j    # branch pad value -c/k
VB3, VKO, VK3, VC3, VCOP = 16, 17, 18, 19, 20
VKPF, VCPF, VSE1B, VSE2B, VNCPF = 21, 22, 23, 24, 25
NV = 26


def _perm():
    """ZS slice order: s=0 is spx[7] (channels 112:128), s>=1 is spx[s-1]."""
    p = np.zeros(128, np.int64)
    for s in range(8):
        for c in range(16):
            p[16 * s + c] = 112 + c if s == 0 else 16 * (s - 1) + c
    return p


def _bnkc(p):
    g, b, m, v = [np.asarray(a, np.float64) for a in p]
    k = g / np.sqrt(v + EPS)
    return k, b - m * k


def _q8(a):
    return np.asarray(a, np.float32).astype(E4).astype(np.float64)


def _prep(inp, T):
    f = lambda n: np.asarray(inp[n], np.float64)
    w1, b1, bn1 = f("w1"), f("b1"), f("bn1")
    cw, cb, ibn = f("cw"), f("cb"), f("ibn")
    w3, b3, bn3 = f("w3"), f("b3"), f("bn3")
    se1w, se1b = f("se1w"), f("se1b")
    se2w, se2b = f("se2w"), f("se2b")
    obn, fbn = f("obn"), f("fbn")
    fcw, fcb = f("fcw"), f("fcb")
    ws = [f(f"ws{j}") for j in range(NIN)]
    perm = _perm()
    kf, cf = _bnkc(fbn)  # (1024,)

    LW8 = np.zeros((NOUT, 128, NDR, 2, 128), np.float64)
    LWF = np.zeros((NOUT, 128, 2, 128), np.float64)
    VEC = np.zeros((NOUT, 128, NV), np.float64)
    TCPF = np.zeros((128, NES * BL), np.float64)

    def put_pair(i, slot, Wm):
        """plane0 = fp8(W), plane1 = fp8(W - plane0)."""
        p0 = _q8(Wm)
        LW8[i, :, slot, 0] = p0
        LW8[i, :, slot, 1] = _q8(Wm - p0)

    for i in range(NOUT):
        k1, c1 = _bnkc(bn1[i])
        k1p, c1p = k1[perm], c1[perm]
        kb, cbv = [], []
        for j in range(NIN):
            kj, cj = _bnkc(ibn[i, j])
            kb.append(kj)
            cbv.append(cj)
        assert np.abs(k1).min() > 1e-3
        assert min(np.abs(k).min() for k in kb) > 1e-3

        def row_kc(j, k1p=k1p, c1p=c1p, kb=kb, cbv=cbv):
            k = k1p.copy()
            c = c1p.copy()
            if j > 0:
                n = 16 * (j + 1)
                k[:n] = np.tile(kb[j - 1], j + 1)
                c[:n] = np.tile(cbv[j - 1], j + 1)
            return k, c

        put_pair(i, SC1A, w1[i][perm, :].T)
        ko, co = _bnkc(obn[i])
        if i > 0:
            # OB is stored pre-scaled by ko_prev, so no fold here
            put_pair(i, SC1B, w1[i][perm, :].T)

        for j in range(NIN):
            krow, crow = row_kc(j)
            blk = [cw[i, j, :, :, d, 0].T for d in range(3)]  # [ci, co]
            tapm = [np.zeros((128, 128)) for _ in range(3)]
            bias_full = np.zeros(128)
            for s in range(j + 2):
                r0 = 16 * s
                const_s = np.zeros(16)
                for d in range(3):
                    tapm[d][r0:r0 + 16, r0:r0 + 16] = \
                        blk[d] * krow[r0:r0 + 16, None]
                    const_s += blk[d].T @ crow[r0:r0 + 16]
                bias_full[r0:r0 + 16] = cb[i, j] + const_s
            # slot A: (tap-2, tap0); slot B: (tap+2, err(tap+2))
            LW8[i, :, STAPA(j), 0] = _q8(tapm[0])
            LW8[i, :, STAPA(j), 1] = _q8(tapm[1])
            put_pair(i, STAPB(j), tapm[2])
            VEC[i, :, VBRB(j)] = bias_full
            VEC[i, :16 * (j + 2), VBRP(j)] = \
                np.tile(-cbv[j] / kb[j], j + 2)

        # conv3 terms, packed as slab pairs (no err compensation)
        c3const = np.zeros(128)
        IX7 = np.zeros((128, 128))
        IX7[0:16, :] = w3[i][:, 112:128].T * k1p[0:16, None]
        c3const += w3[i][:, 112:128] @ c1p[0:16]
        IC3s = []
        for j in range(NIN):
            wj = ws[j][i]
            IC3 = np.zeros((128, 128))
            for s in range(j + 2):
                r0 = 16 * s
                IC3[r0:r0 + 16, :] = \
                    (wj[s] * w3[i][:, 16 * j:16 * j + 16].T) * kb[j][:, None]
                c3const += wj[s] * (w3[i][:, 16 * j:16 * j + 16] @ cbv[j])
            IC3s.append(IC3)
        # pair m reads (slab0-term, slab1-term) at the post(2m) epoch
        pairs3 = [(IX7, IC3s[0]), (IC3s[1], IC3s[2]),
                  (IC3s[3], IC3s[4]), (IC3s[5], IC3s[6])]
        for m, (pa, pb) in enumerate(pairs3):
            LW8[i, :, SC3P(m), 0] = _q8(pa)
            LW8[i, :, SC3P(m), 1] = _q8(pb)

        k3, c3 = _bnkc(bn3[i])
        LWF[i, :, 0, 0:16] = (se1w[i] * k3[None, :] / T).T
        LWF[i, 0:16, 1, :] = se2w[i].T

        VEC[i, :, VB1] = b1[i][perm]
        if i > 0:
            _, co_p = _bnkc(obn[i - 1])
            VEC[i, :, VB1] += (w1[i] @ co_p)[perm]
            VEC[i, :, VCOP] = ko * co_p   # obias' carries the ko prescale
        VEC[i, :, VPAD1] = -c1p / k1p
        VEC[i, :, VB3] = b3[i] + c3const
        VEC[i, :, VKO] = ko
        VEC[i, :, VK3], VEC[i, :, VC3] = k3, ko * c3
        kfi = kf[128 * i:128 * (i + 1)]
        cfi = cf[128 * i:128 * (i + 1)]
        VEC[i, :, VKPF] = kfi           # OB' = ko*OB already
        VEC[i, :, VCPF] = kfi * co + cfi
        VEC[i, :, VNCPF] = -(kfi * co + cfi)
        for bb in (1, 3):  # only max-form (DVE) pool columns need +T*cpf
            TCPF[:, BL * i + bb] = T * (kfi * co + cfi)
        VEC[i, :16, VSE1B] = se1b[i] + se1w[i] @ c3
        VEC[i, :, VSE2B] = se2b[i]
    TCPF[:, 30:32] = (T * cf[896:1024])[:, None]  # x7 b2,b3 use max-form
    vec7 = np.stack([kf[896:1024], cf[896:1024], -cf[896:1024]], axis=1)
    fcwp = np.zeros((128, 8, 2))
    for g in range(8):
        fcwp[:, g, :] = (fcw[:, 128 * g:128 * (g + 1)] / T).T
    lw8 = np.ascontiguousarray(
        LW8.reshape(NOUT, 128, NDR * 256).astype(E4))
    lwf = np.ascontiguousarray(
        LWF.reshape(NOUT, 128, 256).astype(np.float16))
    vec = np.ascontiguousarray(VEC.astype(np.float32))
    return (lw8, lwf, vec,
            np.ascontiguousarray(TCPF.astype(np.float32)),
            np.ascontiguousarray(vec7.astype(np.float32)),
            np.ascontiguousarray(fcwp.reshape(128, 16).astype(np.float16)),
            np.ascontiguousarray(fcb.reshape(2, 1).astype(np.float32)))


def _split_waits(nc, max_waits=1):
    """walrus's TRN2 codegen rejects >1 sync wait on drain/matmul (and
    possibly other) instructions; peel extras onto preceding single-wait
    no-ops on the same engine."""
    n_new = 0
    for fn in nc.m.functions:
        for bb in fn.blocks:
            out = []
            for ins in bb.instructions:
                si = ins.sync_info
                if si is not None and len(si.on_wait) > max_waits:
                    waits = list(si.on_wait)
                    for w in waits[max_waits:]:
                        nop = mybir.InstNoOp(
                            name=f"I-splitwait-{n_new}",
                            sync_info=mybir.SyncInfo(on_wait=[w], on_update=[]),
                            bass_nofuse=True,
                            engine=ins.engine,
                        )
                        out.append(nop)
                        n_new += 1
                    ins.sync_info = mybir.SyncInfo(
                        on_wait=waits[:max_waits], on_update=list(si.on_update))
                out.append(ins)
            bb.instructions = out
    return n_new


def build(T=T0):
    nc = bass.Bass("TRN2")
    P = T + 4
    W2 = BL * P + 4
    lo, hi = 2, 2 + BL * P
    chunks = []
    c = lo
    while c < hi:
        w = min(512, hi - c)
        chunks.append((c, w))
        c += w
    NCH = len(chunks)
    # valid-column pieces (for bn3 squeeze accumulation)
    pieces = []
    for b in range(BL):
        v0, v1 = 4 + P * b, 4 + P * b + T
        for ci, (c0, w) in enumerate(chunks):
            s, e = max(v0, c0), min(v1, c0 + w)
            if s < e:
                pieces.append((ci, s, e - s, b))
    NP = len(pieces)
    assert NP == 2 * BL and all(p[3] == k // 2 for k, p in enumerate(pieces))

    x_d = nc.dram_tensor("x", [BL, CIN, T], F8, kind="ExternalInput")
    xr_d = nc.dram_tensor("xr", [BL, 128, T], F16, kind="ExternalInput")
    lw8_d = nc.dram_tensor("lw8", [NOUT, 128, NDR * 256], F8,
                           kind="ExternalInput")
    lwf_d = nc.dram_tensor("lwf", [NOUT, 128, 256], F16, kind="ExternalInput")
    vec_d = nc.dram_tensor("vec", [NOUT, 128, NV], F32, kind="ExternalInput")
    tcpf_d = nc.dram_tensor("tcpf", [128, NES * BL], F32, kind="ExternalInput")
    vec7_d = nc.dram_tensor("vec7", [128, 3], F32, kind="ExternalInput")
    fcw_d = nc.dram_tensor("fcw", [128, 16], F16, kind="ExternalInput")
    fcb_d = nc.dram_tensor("fcb", [2, 1], F32, kind="ExternalInput")
    y_d = nc.dram_tensor("y", [2, BL], F32, kind="ExternalOutput")

    def pads_ap(t, nact, slab=None):
        """All guard/pad columns: 4-col blocks at 0, P, 2P, 3P, 4P."""
        a = t[:]
        off = a.offset + (0 if slab is None else slab * W2)
        return RAP(a.tensor, off,
                   [[int(a.ap[0][0]), nact], [P, BL + 1], [1, 4]])

    def pair(t, base, stride, w, np_=128, slab=None):
        a = t[:]
        off = a.offset + base + (0 if slab is None else slab * W2)
        return RAP(a.tensor, off,
                   [[int(a.ap[0][0]), np_], [stride, 2], [1, w]])

    with tile.TileContext(nc) as tc:
        with tc.tile_pool(name="state", bufs=1) as state, \
             tc.tile_pool(name="spxp", bufs=2) as spxp, \
             tc.tile_pool(name="wp", bufs=2) as wp, \
             tc.tile_pool(name="vp", bufs=2) as vp, \
             tc.tile_pool(name="bpsum", bufs=2, space="PSUM") as bpsum, \
             tc.tile_pool(name="cpsum", bufs=NCH, space="PSUM") as cpsum, \
             tc.tile_pool(name="spsum", bufs=1, space="PSUM") as spsum:

            ZS2 = state.tile([128, 2, W2], F8, tag="ZS2")
            OB = state.tile([128, W2], F8, tag="OB")
            Rb = state.tile([128, W2], F16, tag="Rb")
            O3 = state.tile([128, W2], F16, tag="O3")
            scrp = state.tile([128, BL, T], F16, tag="scrp")
            sq8 = state.tile([128, NP], F32, tag="sq8")
            sq_r = state.tile([128, BL], F16, tag="sq_r")
            seh = state.tile([128, BL], F16, tag="seh")
            gate = state.tile([128, BL], F32, tag="gate")
            k3g = state.tile([128, BL], F32, tag="k3g")
            obias = state.tile([128, BL], F32, tag="obias")
            Mt = state.tile([128, NES * BL], F32, tag="Mt")
            Mt_r = state.tile([128, NES * BL], F16, tag="Mt_r")
            zero16 = state.tile([128, 1], F16, tag="z16")
            outs = state.tile([2, BL], F32, tag="outs")
            fcw_s = state.tile([128, 16], F16, tag="fcw")
            fcb_s = state.tile([2, 1], F32, tag="fcb")
            tcpf_s = state.tile([128, NES * BL], F32, tag="tcpf")
            vec7_s = state.tile([128, 3], F32, tag="vec7")

            # conv3 slab pairs read all 128 slab1 rows (zero weights on
            # unused rows) before the first block fully populates them
            nc.gpsimd.memset(ZS2[32:64, 1, :], 0)
            nc.gpsimd.memset(ZS2[64:128, 1, :], 0)
            nc.gpsimd.memset(OB[:], 0)
            nc.vector.memset(zero16[:], 0)
            nc.sync.dma_start(out=fcw_s[:], in_=fcw_d[:])
            nc.sync.dma_start(out=fcb_s[:], in_=fcb_d[:])
            nc.sync.dma_start(out=vec7_s[:], in_=vec7_d[:])
            nc.sync.dma_start(out=tcpf_s[:], in_=tcpf_d[:])
            # residual for block 0 = x[:, 0:128] in fp16
            nc.sync.dma_start(
                out=Rb[:, lo:hi].rearrange("p (b q) -> p b q", q=P)[:, :, 2:T + 2],
                in_=xr_d.rearrange("b c t -> c b t"))

            def load_spx(i):
                t = spxp.tile([128, W2], F8, tag="spx")
                nc.gpsimd.memset(pads_ap(t, 128), 0)
                nc.sync.dma_start(
                    out=t[:, lo:hi].rearrange("p (b q) -> p b q", q=P)[:, :, 2:T + 2],
                    in_=x_d[:, 128 * i:128 * (i + 1), :].rearrange("b c t -> c b t"))
                return t

            def load_w(i):
                t = wp.tile([128, NDR, 256], F8, tag="lw8")
                nc.sync.dma_start(
                    out=t[:], in_=lw8_d[i].rearrange("k (s m) -> k s m", m=256))
                tf = wp.tile([128, 2, 128], F16, tag="lwf")
                nc.sync.dma_start(
                    out=tf[:], in_=lwf_d[i].rearrange("k (s m) -> k s m", m=128))
                v = vp.tile([128, NV], F32, tag="vec")
                nc.sync.dma_start(out=v[:], in_=vec_d[i])
                return t, tf, v

            # post-op engine rotation.  GPSIMD cannot read PSUM on HW, so
            # PSUM drains go on ACT/DVE only (ACT is faster per column).
            _rr = [0]
            _pat = "aadadadadadad"  # 8 ACT : 5 DVE

            def post_chunk(dst, ps, nact, biasap, accum=None):
                e = _pat[_rr[0] % len(_pat)]
                _rr[0] += 1
                if e == "a":
                    nc.scalar.activation(dst, ps, ACTF.Relu, bias=biasap,
                                         scale=1.0, accum_out=accum)
                else:
                    if accum is None:
                        nc.vector.tensor_scalar(dst, ps, biasap, 0.0,
                                                ALU.add, ALU.max)
                    else:
                        nc.vector.scalar_tensor_tensor(
                            dst, ps, biasap,
                            zero16[:nact, 0:1].to_broadcast(list(dst.shape)),
                            ALU.add, ALU.max, accum_out=accum)

            def pad_blk(t, nact, Vt, col, blkidx, slab):
                a = t[:]
                off = a.offset + slab * W2 + P * blkidx
                ap = RAP(a.tensor, off, [[int(a.ap[0][0]), nact], [1, 4]])
                bc = Vt[:nact, col:col + 1].to_broadcast([nact, 4])
                nc.gpsimd.tensor_copy(out=ap, in_=bc)

            def post_pass(dst, psl, nact, bias_col, Vt, padt=None,
                          pad_col=None, pad_slab=0):
                biasap = Vt[:nact, bias_col:bias_col + 1]
                for k, (c0, w) in enumerate(chunks):
                    post_chunk(dst[:nact, c0:c0 + w], psl[k][:nact, :w],
                               nact, biasap)
                    if padt is not None:
                        pad_blk(padt, nact, Vt, pad_col, k, pad_slab)

            def w8ap(Wt, slot, nact=128):
                return Wt[:, slot, :].rearrange("k (a m) -> k a m", a=2)[:, :, 0:nact]

            spx = load_spx(0)
            LWt, LWf, Vt = load_w(0)
            pend_c1 = None
            deferred = []  # off-critical ops from the previous block's tail

            for i in range(NOUT):
                nxt = load_spx(i + 1)
                nW = load_w(i + 1) if i < NOUT - 1 else None

                # conv1 psum: spx part was issued in block i-1's tail
                if i == 0:
                    c1ps = [cpsum.tile([128, 512], F32, tag="cps",
                                       name="c1ps")[:, :w] for (c0, w) in chunks]
                    for k, (c0, w) in enumerate(chunks):
                        nc.tensor.matmul(c1ps[k], w8ap(LWt, SC1A),
                                         pair(spx, c0, 0, w),
                                         start=True, stop=True, perf_mode=DR)
                else:
                    c1ps = pend_c1  # completed (SC1B) in prev block's tail
                zs0 = ZS2[:, 0, :]
                post_pass(zs0, c1ps, 128, VB1, Vt,
                          padt=ZS2, pad_col=VPAD1, pad_slab=0)
                # pristine rows needed by odd branches on slab 1 (incl pads)
                for r0 in (32, 64, 96):
                    nc.sync.dma_start(out=ZS2[r0:r0 + 16, 1, :],
                                      in_=ZS2[r0:r0 + 16, 0, :])

                # conv3 accumulation group (filled by slab pairs below)
                cps = [cpsum.tile([128, 512], F32, tag="cps", name="cps")[:, :w]
                       for (c0, w) in chunks]

                # inner scale branches, in place in ZS
                for j in range(NIN):
                    nact = 16 * (j + 2)
                    rj, wj = j % 2, (j + 1) % 2
                    bps = [bpsum.tile([128, 512], F32, tag="bps",
                                      name="bps")[:, :w] for (c0, w) in chunks]
                    # chunk-pair interleave: with bufs=2 PSUM banks, group k
                    # must close (TAPB) before bank reuse at chunk k+2
                    for kk in range(0, NCH, 2):
                        ks = [k for k in (kk, kk + 1) if k < NCH]
                        for k in ks:
                            c0, w = chunks[k]
                            nc.tensor.matmul(bps[k][:nact, :],
                                             w8ap(LWt, STAPA(j), nact),
                                             pair(ZS2, c0 - 2, 2, w, slab=rj),
                                             start=True, stop=False,
                                             perf_mode=DR)
                        for k in ks:
                            c0, w = chunks[k]
                            nc.tensor.matmul(bps[k][:nact, :],
                                             w8ap(LWt, STAPB(j), nact),
                                             pair(ZS2, c0 + 2, 0, w, slab=rj),
                                             start=False, stop=True,
                                             perf_mode=DR)
                    post_pass(ZS2[:, wj, :], bps, nact, VBRB(j), Vt,
                              padt=ZS2, pad_col=VBRP(j), pad_slab=wj)
                    if deferred:
                        deferred.pop(0)()
                    if j % 2 == 0:
                        # conv3 slab pair m=j//2: (slab0 term, slab1 term)
                        for k, (c0, w) in enumerate(chunks):
                            nc.tensor.matmul(cps[k], w8ap(LWt, SC3P(j // 2)),
                                             pair(ZS2, c0, W2, w),
                                             start=(j == 0), stop=(j == 6),
                                             perf_mode=DR)

                while deferred:
                    deferred.pop(0)()
                # bn3 relu -> O3 (fp16), per-piece accum for SE squeeze
                for kpc, (ci, s, w, b) in enumerate(pieces):
                    c0 = chunks[ci][0]
                    post_chunk(O3[:, s:s + w], cps[ci][:, s - c0:s - c0 + w],
                               128, Vt[:, VB3:VB3 + 1],
                               accum=sq8[:, kpc:kpc + 1])
                # next block's conv1 spx-part (fills the SE gap)
                if nW is not None:
                    pend_c1 = [cpsum.tile([128, 512], F32, tag="cps",
                                          name="c1ps")[:, :w]
                               for (c0, w) in chunks]
                    for k, (c0, w) in enumerate(chunks):
                        nc.tensor.matmul(pend_c1[k], w8ap(nW[0], SC1A),
                                         pair(nxt, c0, 0, w),
                                         start=True, stop=False, perf_mode=DR)

                # SE squeeze/excite (fp16)
                nc.vector.tensor_tensor(
                    sq_r[:], sq8[:, 0:NP:2], sq8[:, 1:NP:2], ALU.add)
                ps1 = spsum.tile([128, BL], F32, tag="sps")
                nc.tensor.matmul(ps1, LWf[:, 0, :], sq_r[:],
                                 start=True, stop=True)
                nc.scalar.activation(seh[:], ps1, ACTF.Relu,
                                     bias=Vt[:, VSE1B:VSE1B + 1], scale=1.0)
                ps2 = spsum.tile([128, BL], F32, tag="sps")
                nc.tensor.matmul(ps2, LWf[:, 1, :], seh[:],
                                 start=True, stop=True)
                nc.scalar.activation(gate[:], ps2, ACTF.Sigmoid,
                                     bias=Vt[:, VSE2B:VSE2B + 1], scale=1.0)
                # gate-scaled scalars: k3g = gate*k3, obias = gate*c3 + co_prev
                nc.vector.tensor_tensor(
                    k3g[:], gate[:],
                    Vt[:, VK3:VK3 + 1].to_broadcast([128, BL]), ALU.mult)
                nc.vector.scalar_tensor_tensor(
                    obias[:], gate[:], Vt[:, VC3:VC3 + 1],
                    Vt[:, VCOP:VCOP + 1].to_broadcast([128, BL]),
                    ALU.mult, ALU.add)

                # tail per batch: u = O3*k3g + res (in place);
                # OB = relu(u + obias); pool accum; Rb = OB*ko + nxt
                c1b_chunks = {0: [0], 1: [1], 2: [2], 3: [3, 4]}
                for b in range(BL):
                    v0 = 4 + P * b
                    o3b = O3[:, v0:v0 + T]
                    nc.vector.scalar_tensor_tensor(
                        o3b, o3b, k3g[:, b:b + 1], Rb[:, v0:v0 + T],
                        ALU.mult, ALU.add)
                    obb = OB[:, v0:v0 + T]
                    nc.scalar.activation(obb, o3b, ACTF.Relu,
                                         bias=obias[:, b:b + 1],
                                         scale=Vt[:, VKO:VKO + 1])
                    # complete next block's conv1 psum group on the chunks
                    # this batch unblocks (PE gets work while tail runs)
                    if nW is not None:
                        for k in c1b_chunks[b]:
                            c0, w = chunks[k]
                            nc.tensor.matmul(pend_c1[k], w8ap(nW[0], SC1B),
                                             pair(OB, c0, 0, w),
                                             start=False, stop=True,
                                             perf_mode=DR)
                # pooling + residual build are off the critical path:
                # defer them into the next block's branch slots so they sit
                # behind the critical posts in the engine queues
                def mk_pool(b, i=i, Vt=Vt):
                    v0 = 4 + P * b
                    col = BL * i + b
                    if b % 2 == 0:
                        return lambda: nc.scalar.activation(
                            scrp[:, b, :], OB[:, v0:v0 + T], ACTF.Relu,
                            bias=Vt[:, VCPF:VCPF + 1],
                            scale=Vt[:, VKPF:VKPF + 1],
                            accum_out=Mt[:, col:col + 1])
                    return lambda: nc.vector.scalar_tensor_tensor(
                        scrp[:, b, :], OB[:, v0:v0 + T],
                        Vt[:, VKPF:VKPF + 1],
                        Vt[:, VNCPF:VNCPF + 1].to_broadcast([128, T]),
                        ALU.mult, ALU.max, accum_out=Mt[:, col:col + 1])

                def mk_rb(h, nxt=nxt, Vt=Vt):
                    rbv = Rb[:, lo:hi].rearrange("p (b q) -> p b q", q=P)
                    obv = OB[:, lo:hi].rearrange("p (b q) -> p b q", q=P)
                    nxv = nxt[:, lo:hi].rearrange("p (b q) -> p b q", q=P)
                    sl = slice(2 * h, 2 * h + 2)
                    return lambda: nc.vector.tensor_tensor(
                        rbv[:, sl, 2:T + 2], obv[:, sl, 2:T + 2],
                        nxv[:, sl, 2:T + 2], ALU.add)

                deferred = [mk_pool(0), mk_pool(1)]
                if i < NOUT - 1:
                    deferred += [mk_rb(0), mk_rb(1)]
                deferred += [mk_pool(2), mk_pool(3)]
                if i == NOUT - 1:
                    while deferred:
                        deferred.pop(0)()
                spx = nxt
                if nW is not None:
                    LWt, LWf, Vt = nW

            # final-pool contribution of raw spx[7]: relu(kf*x + cf)
            sxv = spx[:, lo:hi].rearrange("p (b q) -> p b q", q=P)
            for b in range(BL):
                col = BL * 7 + b
                if b < 2:
                    nc.scalar.activation(scrp[:, b, :], sxv[:, b, 2:T + 2],
                                         ACTF.Relu, bias=vec7_s[:, 1:2],
                                         scale=vec7_s[:, 0:1],
                                         accum_out=Mt[:, col:col + 1])
                else:
                    nc.vector.scalar_tensor_tensor(
                        scrp[:, b, :], sxv[:, b, 2:T + 2], vec7_s[:, 0:1],
                        vec7_s[:, 2:3].to_broadcast([128, T]),
                        ALU.mult, ALU.max, accum_out=Mt[:, col:col + 1])

            # classifier: y = sum_g fcw_g.T @ M_g + fcb
            nc.vector.tensor_tensor(Mt_r[:], Mt[:], tcpf_s[:], ALU.add)
            fps = spsum.tile([2, BL], F32, tag="sps", name="fps")
            for g in range(8):
                nc.tensor.matmul(fps, fcw_s[:, 2 * g:2 * g + 2],
                                 Mt_r[:, BL * g:BL * (g + 1)],
                                 start=(g == 0), stop=(g == 7))
            nc.scalar.activation(outs[:], fps, ACTF.Identity,
                                 bias=fcb_s[:], scale=1.0)
            nc.sync.dma_start(out=y_d[:], in_=outs[:])

    return nc


_NC_CACHE = {}


def _get_nc(T):
    if T not in _NC_CACHE:
        nc = build(T)
        _split_waits(nc)
        _NC_CACHE[T] = nc
    return _NC_CACHE[T]


def make_in_maps(inputs):
    x = np.asarray(inputs["x"], np.float32)
    T = x.shape[2]
    lw8, lwf, vec, tcpf, vec7, fcw, fcb = _prep(inputs, T)
    x8 = np.ascontiguousarray(x.astype(E4))
    xr = np.ascontiguousarray(x[:, 0:128, :].astype(np.float16))
    in_maps = []
    for core in range(NCORES):
        in_maps.append({
            "x": np.ascontiguousarray(x8[core * BL:(core + 1) * BL]),
            "xr": np.ascontiguousarray(xr[core * BL:(core + 1) * BL]),
            "lw8": lw8, "lwf": lwf, "vec": vec, "tcpf": tcpf,
            "vec7": vec7, "fcw": fcw, "fcb": fcb,
        })
    return in_maps, T


def kernel(**inputs):
    in_maps, T = make_in_maps(inputs)
    nc = _get_nc(T)
    res = run_bass_kernel_spmd(nc, in_maps, list(range(NCORES)))
    out = np.concatenate(
        [np.asarray(res.results[c]["y"]).T for c in range(NCORES)], axis=0)
    return np.ascontiguousarray(out.astype(np.float32))


# revision 16
# speedup vs baseline: 1.2419x; 1.0044x over previous
"""Trainium2 Bass kernel for nn_Nested_Res2Net_TDNN (B=32, CIN=1024, T=600).

Sharding: data-parallel over batch across 8 NeuronCores (4 per core),
parameters replicated.

v2: fp8(e4m3) DoubleRow matmuls.  Data stays channels-on-128-partitions,
(batch x time + guard pads) on the free dim, exactly as the fp16 version;
every conv becomes fp8 DoubleRow pairs at 0.5 cycles/column:
  - the 3 dilated taps of each scale branch pair as (tap-2, tap0) via a
    stride-2 rhs access pattern, plus (tap+2, fp8-error-compensation) via
    a stride-0 pair,
  - the 1x1 convs (conv1 spx/OB parts, conv3 terms) pair each fp8 weight
    plane with its own fp8 quantization-error plane (stride-0 rhs), which
    recovers most of the weight-quantization error for free,
  - BN affines fold into following-matmul weights (rows scaled by k) with
    -c/k guard-pad values so dilated taps see exact zero padding,
  - post-matmul relu/bias writes are spread over ACT + DVE + GPSIMD,
  - the SE/tail is fused with scalar_tensor_tensor: u=(z3*k3g)+res in
    place, OB=relu(u+gate*c3+co_prev), pooling rides as
    (OB*kpf) max (-cpf) with accum_out, residual = (OB*ko)+spx_next.
fp16 is kept for the SE squeeze path, O3/residual tensors and the
classifier; PSUM is fp32 throughout.
"""

import sys

for _p in ("/opt/trn_rl_repo",):
    if _p not in sys.path:
        sys.path.insert(0, _p)

import numpy as np
import ml_dtypes

import concourse.bass as bass
import concourse.mybir as mybir
import concourse.tile as tile
from concourse.bass_utils import run_bass_kernel_spmd
from bass_rust import AP as RAP

B, CIN, T0 = 32, 1024, 600
NES, SCALE = 8, 8
CBLK, WSC = 128, 16
NIN, NOUT = 7, 7
EPS = 1e-5
NCORES = 8
BL = B // NCORES

F32 = mybir.dt.float32
F16 = mybir.dt.float16
F8 = mybir.dt.float8e4
E4 = ml_dtypes.float8_e4m3
ALU = mybir.AluOpType
ACTF = mybir.ActivationFunctionType
DR = mybir.MatmulPerfMode.DoubleRow

# DoubleRow weight slots (each [K=128, 2 planes, 128 out]) per block
SC1A = 0                      # conv1 spx part: (W1p, err)
SC1B = 1                      # conv1 OB part: (W1B, err)
STAPA = lambda j: 2 + 2 * j   # branch j taps (-2, 0)
STAPB = lambda j: 3 + 2 * j   # branch j (tap +2, err)
SC3P = lambda m: 16 + m       # conv3 slab pairs: (IX7,c0),(c1,c2),(c3,c4),(c5,c6)
NDR = 20

# VEC columns (fp32 per-partition scalars)
VB1, VPAD1 = 0, 1
VBRB = lambda j: 2 + 2 * j    # branch post bias
VBRP = lambda j: 3 + 2 * j    # branch pad value -c/k
VB3, VKO, VK3, VC3, VCOP = 16, 17, 18, 19, 20
VKPF, VCPF, VSE1B, VSE2B, VNCPF = 21, 22, 23, 24, 25
NV = 26


def _perm():
    """ZS slice order: s=0 is spx[7] (channels 112:128), s>=1 is spx[s-1]."""
    p = np.zeros(128, np.int64)
    for s in range(8):
        for c in range(16):
            p[16 * s + c] = 112 + c if s == 0 else 16 * (s - 1) + c
    return p


def _bnkc(p):
    g, b, m, v = [np.asarray(a, np.float64) for a in p]
    k = g / np.sqrt(v + EPS)
    return k, b - m * k


def _q8(a):
    return np.asarray(a, np.float32).astype(E4).astype(np.float64)


def _prep(inp, T):
    f = lambda n: np.asarray(inp[n], np.float64)
    w1, b1, bn1 = f("w1"), f("b1"), f("bn1")
    cw, cb, ibn = f("cw"), f("cb"), f("ibn")
    w3, b3, bn3 = f("w3"), f("b3"), f("bn3")
    se1w, se1b = f("se1w"), f("se1b")
    se2w, se2b = f("se2w"), f("se2b")
    obn, fbn = f("obn"), f("fbn")
    fcw, fcb = f("fcw"), f("fcb")
    ws = [f(f"ws{j}") for j in range(NIN)]
    perm = _perm()
    kf, cf = _bnkc(fbn)  # (1024,)

    LW8 = np.zeros((NOUT, 128, NDR, 2, 128), np.float64)
    LWF = np.zeros((NOUT, 128, 2, 128), np.float64)
    VEC = np.zeros((NOUT, 128, NV), np.float64)
    TCPF = np.zeros((128, NES * BL), np.float64)

    def put_pair(i, slot, Wm):
        """plane0 = fp8(W), plane1 = fp8(W - plane0)."""
        p0 = _q8(Wm)
        LW8[i, :, slot, 0] = p0
        LW8[i, :, slot, 1] = _q8(Wm - p0)

    for i in range(NOUT):
        k1, c1 = _bnkc(bn1[i])
        k1p, c1p = k1[perm], c1[perm]
        kb, cbv = [], []
        for j in range(NIN):
            kj, cj = _bnkc(ibn[i, j])
            kb.append(kj)
            cbv.append(cj)
        assert np.abs(k1).min() > 1e-3
        assert min(np.abs(k).min() for k in kb) > 1e-3

        def row_kc(j, k1p=k1p, c1p=c1p, kb=kb, cbv=cbv):
            k = k1p.copy()
            c = c1p.copy()
            if j > 0:
                n = 16 * (j + 1)
                k[:n] = np.tile(kb[j - 1], j + 1)
                c[:n] = np.tile(cbv[j - 1], j + 1)
            return k, c

        put_pair(i, SC1A, w1[i][perm, :].T)
        ko, co = _bnkc(obn[i])
        if i > 0:
            # OB is stored pre-scaled by ko_prev, so no fold here
            put_pair(i, SC1B, w1[i][perm, :].T)

        for j in range(NIN):
            krow, crow = row_kc(j)
            blk = [cw[i, j, :, :, d, 0].T for d in range(3)]  # [ci, co]
            tapm = [np.zeros((128, 128)) for _ in range(3)]
            bias_full = np.zeros(128)
            for s in range(j + 2):
                r0 = 16 * s
                const_s = np.zeros(16)
                for d in range(3):
                    tapm[d][r0:r0 + 16, r0:r0 + 16] = \
                        blk[d] * krow[r0:r0 + 16, None]
                    const_s += blk[d].T @ crow[r0:r0 + 16]
                bias_full[r0:r0 + 16] = cb[i, j] + const_s
            # slot A: (tap-2, tap0); slot B: (tap+2, err(tap+2))
            LW8[i, :, STAPA(j), 0] = _q8(tapm[0])
            LW8[i, :, STAPA(j), 1] = _q8(tapm[1])
            put_pair(i, STAPB(j), tapm[2])
            VEC[i, :, VBRB(j)] = bias_full
            VEC[i, :16 * (j + 2), VBRP(j)] = \
                np.tile(-cbv[j] / kb[j], j + 2)

        # conv3 terms, packed as slab pairs (no err compensation)
        c3const = np.zeros(128)
        IX7 = np.zeros((128, 128))
        IX7[0:16, :] = w3[i][:, 112:128].T * k1p[0:16, None]
        c3const += w3[i][:, 112:128] @ c1p[0:16]
        IC3s = []
        for j in range(NIN):
            wj = ws[j][i]
            IC3 = np.zeros((128, 128))
            for s in range(j + 2):
                r0 = 16 * s
                IC3[r0:r0 + 16, :] = \
                    (wj[s] * w3[i][:, 16 * j:16 * j + 16].T) * kb[j][:, None]
                c3const += wj[s] * (w3[i][:, 16 * j:16 * j + 16] @ cbv[j])
            IC3s.append(IC3)
        # pair m reads (slab0-term, slab1-term) at the post(2m) epoch
        pairs3 = [(IX7, IC3s[0]), (IC3s[1], IC3s[2]),
                  (IC3s[3], IC3s[4]), (IC3s[5], IC3s[6])]
        for m, (pa, pb) in enumerate(pairs3):
            LW8[i, :, SC3P(m), 0] = _q8(pa)
            LW8[i, :, SC3P(m), 1] = _q8(pb)

        k3, c3 = _bnkc(bn3[i])
        LWF[i, :, 0, 0:16] = (se1w[i] * k3[None, :] / T).T
        LWF[i, 0:16, 1, :] = se2w[i].T

        VEC[i, :, VB1] = b1[i][perm]
        if i > 0:
            _, co_p = _bnkc(obn[i - 1])
            VEC[i, :, VB1] += (w1[i] @ co_p)[perm]
            VEC[i, :, VCOP] = ko * co_p   # obias' carries the ko prescale
        VEC[i, :, VPAD1] = -c1p / k1p
        VEC[i, :, VB3] = b3[i] + c3const
        VEC[i, :, VKO] = ko
        VEC[i, :, VK3], VEC[i, :, VC3] = k3, ko * c3
        kfi = kf[128 * i:128 * (i + 1)]
        cfi = cf[128 * i:128 * (i + 1)]
        VEC[i, :, VKPF] = kfi           # OB' = ko*OB already
        VEC[i, :, VCPF] = kfi * co + cfi
        VEC[i, :, VNCPF] = -(kfi * co + cfi)
        for bb in (1, 3):  # only max-form (DVE) pool columns need +T*cpf
            TCPF[:, BL * i + bb] = T * (kfi * co + cfi)
        VEC[i, :16, VSE1B] = se1b[i] + se1w[i] @ c3
        VEC[i, :, VSE2B] = se2b[i]
    TCPF[:, 30:32] = (T * cf[896:1024])[:, None]  # x7 b2,b3 use max-form
    vec7 = np.stack([kf[896:1024], cf[896:1024], -cf[896:1024]], axis=1)
    fcwp = np.zeros((128, 8, 2))
    for g in range(8):
        fcwp[:, g, :] = (fcw[:, 128 * g:128 * (g + 1)] / T).T
    lw8 = np.ascontiguousarray(
        LW8.reshape(NOUT, 128, NDR * 256).astype(E4))
    lwf = np.ascontiguousarray(
        LWF.reshape(NOUT, 128, 256).astype(np.float16))
    vec = np.ascontiguousarray(VEC.astype(np.float32))
    return (lw8, lwf, vec,
            np.ascontiguousarray(TCPF.astype(np.float32)),
            np.ascontiguousarray(vec7.astype(np.float32)),
            np.ascontiguousarray(fcwp.reshape(128, 16).astype(np.float16)),
            np.ascontiguousarray(fcb.reshape(2, 1).astype(np.float32)))


def _split_waits(nc, max_waits=1):
    """walrus's TRN2 codegen rejects >1 sync wait on drain/matmul (and
    possibly other) instructions; peel extras onto preceding single-wait
    no-ops on the same engine."""
    n_new = 0
    for fn in nc.m.functions:
        for bb in fn.blocks:
            out = []
            for ins in bb.instructions:
                si = ins.sync_info
                if si is not None and len(si.on_wait) > max_waits:
                    waits = list(si.on_wait)
                    for w in waits[max_waits:]:
                        nop = mybir.InstNoOp(
                            name=f"I-splitwait-{n_new}",
                            sync_info=mybir.SyncInfo(on_wait=[w], on_update=[]),
                            bass_nofuse=True,
                            engine=ins.engine,
                        )
                        out.append(nop)
                        n_new += 1
                    ins.sync_info = mybir.SyncInfo(
                        on_wait=waits[:max_waits], on_update=list(si.on_update))
                out.append(ins)
            bb.instructions = out
    return n_new


def build(T=T0):
    nc = bass.Bass("TRN2")
    P = T + 4
    W2 = BL * P + 4
    lo, hi = 2, 2 + BL * P
    chunks = []
    c = lo
    while c < hi:
        w = min(512, hi - c)
        chunks.append((c, w))
        c += w
    NCH = len(chunks)
    # valid-column pieces (for bn3 squeeze accumulation)
    pieces = []
    for b in range(BL):
        v0, v1 = 4 + P * b, 4 + P * b + T
        for ci, (c0, w) in enumerate(chunks):
            s, e = max(v0, c0), min(v1, c0 + w)
            if s < e:
                pieces.append((ci, s, e - s, b))
    NP = len(pieces)
    assert NP == 2 * BL and all(p[3] == k // 2 for k, p in enumerate(pieces))

    x_d = nc.dram_tensor("x", [BL, CIN, T], F8, kind="ExternalInput")
    xr_d = nc.dram_tensor("xr", [BL, 128, T], F16, kind="ExternalInput")
    lw8_d = nc.dram_tensor("lw8", [NOUT, 128, NDR * 256], F8,
                           kind="ExternalInput")
    lwf_d = nc.dram_tensor("lwf", [NOUT, 128, 256], F16, kind="ExternalInput")
    vec_d = nc.dram_tensor("vec", [NOUT, 128, NV], F32, kind="ExternalInput")
    tcpf_d = nc.dram_tensor("tcpf", [128, NES * BL], F32, kind="ExternalInput")
    vec7_d = nc.dram_tensor("vec7", [128, 3], F32, kind="ExternalInput")
    fcw_d = nc.dram_tensor("fcw", [128, 16], F16, kind="ExternalInput")
    fcb_d = nc.dram_tensor("fcb", [2, 1], F32, kind="ExternalInput")
    y_d = nc.dram_tensor("y", [2, BL], F32, kind="ExternalOutput")

    def pads_ap(t, nact, slab=None):
        """All guard/pad columns: 4-col blocks at 0, P, 2P, 3P, 4P."""
        a = t[:]
        off = a.offset + (0 if slab is None else slab * W2)
        return RAP(a.tensor, off,
                   [[int(a.ap[0][0]), nact], [P, BL + 1], [1, 4]])

    def pair(t, base, stride, w, np_=128, slab=None):
        a = t[:]
        off = a.offset + base + (0 if slab is None else slab * W2)
        return RAP(a.tensor, off,
                   [[int(a.ap[0][0]), np_], [stride, 2], [1, w]])

    with tile.TileContext(nc) as tc:
        with tc.tile_pool(name="state", bufs=1) as state, \
             tc.tile_pool(name="spxp", bufs=2) as spxp, \
             tc.tile_pool(name="wp", bufs=2) as wp, \
             tc.tile_pool(name="vp", bufs=2) as vp, \
             tc.tile_pool(name="bpsum", bufs=2, space="PSUM") as bpsum, \
             tc.tile_pool(name="cpsum", bufs=NCH, space="PSUM") as cpsum, \
             tc.tile_pool(name="spsum", bufs=1, space="PSUM") as spsum:

            ZS2 = state.tile([128, 2, W2], F8, tag="ZS2")
            OB = state.tile([128, W2], F8, tag="OB")
            Rb = state.tile([128, W2], F16, tag="Rb")
            O3 = state.tile([128, W2], F16, tag="O3")
            scrp = state.tile([128, BL, T], F16, tag="scrp")
            sq8 = state.tile([128, NP], F32, tag="sq8")
            sq_r = state.tile([128, BL], F16, tag="sq_r")
            seh = state.tile([128, BL], F16, tag="seh")
            gate = state.tile([128, BL], F32, tag="gate")
            k3g = state.tile([128, BL], F32, tag="k3g")
            obias = state.tile([128, BL], F32, tag="obias")
            Mt = state.tile([128, NES * BL], F32, tag="Mt")
            Mt_r = state.tile([128, NES * BL], F16, tag="Mt_r")
            zero16 = state.tile([128, 1], F16, tag="z16")
            outs = state.tile([2, BL], F32, tag="outs")
            fcw_s = state.tile([128, 16], F16, tag="fcw")
            fcb_s = state.tile([2, 1], F32, tag="fcb")
            tcpf_s = state.tile([128, NES * BL], F32, tag="tcpf")
            vec7_s = state.tile([128, 3], F32, tag="vec7")

            # conv3 slab pairs read all 128 slab1 rows (zero weights on
            # unused rows) before the first block fully populates them
            nc.gpsimd.memset(ZS2[32:64, 1, :], 0)
            nc.gpsimd.memset(ZS2[64:128, 1, :], 0)
            nc.gpsimd.memset(OB[:], 0)
            nc.vector.memset(zero16[:], 0)

            def load_spx(i):
                t = spxp.tile([128, W2], F8, tag="spx")
                nc.gpsimd.memset(pads_ap(t, 128), 0)
                nc.sync.dma_start(
                    out=t[:, lo:hi].rearrange("p (b q) -> p b q", q=P)[:, :, 2:T + 2],
                    in_=x_d[:, 128 * i:128 * (i + 1), :].rearrange("b c t -> c b t"))
                return t

            def load_w(i):
                t = wp.tile([128, NDR, 256], F8, tag="lw8")
                nc.sync.dma_start(
                    out=t[:], in_=lw8_d[i].rearrange("k (s m) -> k s m", m=256))
                tf = wp.tile([128, 2, 128], F16, tag="lwf")
                nc.sync.dma_start(
                    out=tf[:], in_=lwf_d[i].rearrange("k (s m) -> k s m", m=128))
                v = vp.tile([128, NV], F32, tag="vec")
                nc.sync.dma_start(out=v[:], in_=vec_d[i])
                return t, tf, v

            # post-op engine rotation.  GPSIMD cannot read PSUM on HW, so
            # PSUM drains go on ACT/DVE only (ACT is faster per column).
            _rr = [0]
            _pat = "aadadadadadad"  # 8 ACT : 5 DVE

            def post_chunk(dst, ps, nact, biasap, accum=None):
                e = _pat[_rr[0] % len(_pat)]
                _rr[0] += 1
                if e == "a":
                    nc.scalar.activation(dst, ps, ACTF.Relu, bias=biasap,
                                         scale=1.0, accum_out=accum)
                else:
                    if accum is None:
                        nc.vector.tensor_scalar(dst, ps, biasap, 0.0,
                                                ALU.add, ALU.max)
                    else:
                        nc.vector.scalar_tensor_tensor(
                            dst, ps, biasap,
                            zero16[:nact, 0:1].to_broadcast(list(dst.shape)),
                            ALU.add, ALU.max, accum_out=accum)

            def pad_blk(t, nact, Vt, col, blkidx, slab):
                a = t[:]
                off = a.offset + slab * W2 + P * blkidx
                ap = RAP(a.tensor, off, [[int(a.ap[0][0]), nact], [1, 4]])
                bc = Vt[:nact, col:col + 1].to_broadcast([nact, 4])
                nc.gpsimd.tensor_copy(out=ap, in_=bc)

            def post_pass(dst, psl, nact, bias_col, Vt, padt=None,
                          pad_col=None, pad_slab=0):
                biasap = Vt[:nact, bias_col:bias_col + 1]
                for k, (c0, w) in enumerate(chunks):
                    post_chunk(dst[:nact, c0:c0 + w], psl[k][:nact, :w],
                               nact, biasap)
                    if padt is not None:
                        pad_blk(padt, nact, Vt, pad_col, k, pad_slab)

            def w8ap(Wt, slot, nact=128):
                return Wt[:, slot, :].rearrange("k (a m) -> k a m", a=2)[:, :, 0:nact]

            spx = load_spx(0)
            LWt, LWf, Vt = load_w(0)
            # bulky/late-needed loads go after block-0's critical DMAs
            nc.sync.dma_start(
                out=Rb[:, lo:hi].rearrange("p (b q) -> p b q", q=P)[:, :, 2:T + 2],
                in_=xr_d.rearrange("b c t -> c b t"))
            nc.sync.dma_start(out=fcw_s[:], in_=fcw_d[:])
            nc.sync.dma_start(out=fcb_s[:], in_=fcb_d[:])
            nc.sync.dma_start(out=vec7_s[:], in_=vec7_d[:])
            nc.sync.dma_start(out=tcpf_s[:], in_=tcpf_d[:])
            pend_c1 = None
            deferred = []  # off-critical ops from the previous block's tail

            for i in range(NOUT):
                nxt = load_spx(i + 1)
                nW = load_w(i + 1) if i < NOUT - 1 else None

                # conv1 psum: spx part was issued in block i-1's tail
                if i == 0:
                    c1ps = [cpsum.tile([128, 512], F32, tag="cps",
                                       name="c1ps")[:, :w] for (c0, w) in chunks]
                    for k, (c0, w) in enumerate(chunks):
                        nc.tensor.matmul(c1ps[k], w8ap(LWt, SC1A),
                                         pair(spx, c0, 0, w),
                                         start=True, stop=True, perf_mode=DR)
                else:
                    c1ps = pend_c1  # completed (SC1B) in prev block's tail
                zs0 = ZS2[:, 0, :]
                post_pass(zs0, c1ps, 128, VB1, Vt,
                          padt=ZS2, pad_col=VPAD1, pad_slab=0)
                # pristine rows needed by odd branches on slab 1 (incl pads)
                for r0 in (32, 64, 96):
                    nc.sync.dma_start(out=ZS2[r0:r0 + 16, 1, :],
                                      in_=ZS2[r0:r0 + 16, 0, :])

                # conv3 accumulation group (filled by slab pairs below)
                cps = [cpsum.tile([128, 512], F32, tag="cps", name="cps")[:, :w]
                       for (c0, w) in chunks]

                # inner scale branches, in place in ZS
                for j in range(NIN):
                    nact = 16 * (j + 2)
                    rj, wj = j % 2, (j + 1) % 2
                    bps = [bpsum.tile([128, 512], F32, tag="bps",
                                      name="bps")[:, :w] for (c0, w) in chunks]
                    # chunk-pair interleave: with bufs=2 PSUM banks, group k
                    # must close (TAPB) before bank reuse at chunk k+2
                    for kk in range(0, NCH, 2):
                        ks = [k for k in (kk, kk + 1) if k < NCH]
                        for k in ks:
                            c0, w = chunks[k]
                            nc.tensor.matmul(bps[k][:nact, :],
                                             w8ap(LWt, STAPA(j), nact),
                                             pair(ZS2, c0 - 2, 2, w, slab=rj),
                                             start=True, stop=False,
                                             perf_mode=DR)
                        for k in ks:
                            c0, w = chunks[k]
                            nc.tensor.matmul(bps[k][:nact, :],
                                             w8ap(LWt, STAPB(j), nact),
                                             pair(ZS2, c0 + 2, 0, w, slab=rj),
                                             start=False, stop=True,
                                             perf_mode=DR)
                    post_pass(ZS2[:, wj, :], bps, nact, VBRB(j), Vt,
                              padt=ZS2, pad_col=VBRP(j), pad_slab=wj)
                    if deferred:
                        deferred.pop(0)()
                    if j % 2 == 0:
                        # conv3 slab pair m=j//2: (slab0 term, slab1 term)
                        for k, (c0, w) in enumerate(chunks):
                            nc.tensor.matmul(cps[k], w8ap(LWt, SC3P(j // 2)),
                                             pair(ZS2, c0, W2, w),
                                             start=(j == 0), stop=(j == 6),
                                             perf_mode=DR)

                while deferred:
                    deferred.pop(0)()
                # bn3 relu -> O3 (fp16), per-piece accum for SE squeeze
                for kpc, (ci, s, w, b) in enumerate(pieces):
                    c0 = chunks[ci][0]
                    post_chunk(O3[:, s:s + w], cps[ci][:, s - c0:s - c0 + w],
                               128, Vt[:, VB3:VB3 + 1],
                               accum=sq8[:, kpc:kpc + 1])
                # next block's conv1 spx-part (fills the SE gap)
                if nW is not None:
                    pend_c1 = [cpsum.tile([128, 512], F32, tag="cps",
                                          name="c1ps")[:, :w]
                               for (c0, w) in chunks]
                    for k, (c0, w) in enumerate(chunks):
                        nc.tensor.matmul(pend_c1[k], w8ap(nW[0], SC1A),
                                         pair(nxt, c0, 0, w),
                                         start=True, stop=False, perf_mode=DR)

                # SE squeeze/excite (fp16)
                nc.vector.tensor_tensor(
                    sq_r[:], sq8[:, 0:NP:2], sq8[:, 1:NP:2], ALU.add)
                ps1 = spsum.tile([128, BL], F32, tag="sps")
                nc.tensor.matmul(ps1, LWf[:, 0, :], sq_r[:],
                                 start=True, stop=True)
                nc.scalar.activation(seh[:], ps1, ACTF.Relu,
                                     bias=Vt[:, VSE1B:VSE1B + 1], scale=1.0)
                ps2 = spsum.tile([128, BL], F32, tag="sps")
                nc.tensor.matmul(ps2, LWf[:, 1, :], seh[:],
                                 start=True, stop=True)
                nc.scalar.activation(gate[:], ps2, ACTF.Sigmoid,
                                     bias=Vt[:, VSE2B:VSE2B + 1], scale=1.0)
                # gate-scaled scalars: k3g = gate*k3, obias = gate*c3 + co_prev
                nc.vector.tensor_tensor(
                    k3g[:], gate[:],
                    Vt[:, VK3:VK3 + 1].to_broadcast([128, BL]), ALU.mult)
                nc.vector.scalar_tensor_tensor(
                    obias[:], gate[:], Vt[:, VC3:VC3 + 1],
                    Vt[:, VCOP:VCOP + 1].to_broadcast([128, BL]),
                    ALU.mult, ALU.add)

                # tail per batch: u = O3*k3g + res (in place);
                # OB = relu(u + obias); pool accum; Rb = OB*ko + nxt
                c1b_chunks = {0: [0], 1: [1], 2: [2], 3: [3, 4]}
                for b in range(BL):
                    v0 = 4 + P * b
                    o3b = O3[:, v0:v0 + T]
                    nc.vector.scalar_tensor_tensor(
                        o3b, o3b, k3g[:, b:b + 1], Rb[:, v0:v0 + T],
                        ALU.mult, ALU.add)
                    obb = OB[:, v0:v0 + T]
                    nc.scalar.activation(obb, o3b, ACTF.Relu,
                                         bias=obias[:, b:b + 1],
                                         scale=Vt[:, VKO:VKO + 1])
                    # complete next block's conv1 psum group on the chunks
                    # this batch unblocks (PE gets work while tail runs)
                    if nW is not None:
                        for k in c1b_chunks[b]:
                            c0, w = chunks[k]
                            nc.tensor.matmul(pend_c1[k], w8ap(nW[0], SC1B),
                                             pair(OB, c0, 0, w),
                                             start=False, stop=True,
                                             perf_mode=DR)
                # pooling + residual build are off the critical path:
                # defer them into the next block's branch slots so they sit
                # behind the critical posts in the engine queues
                def mk_pool(b, i=i, Vt=Vt):
                    v0 = 4 + P * b
                    col = BL * i + b
                    if b % 2 == 0:
                        return lambda: nc.scalar.activation(
                            scrp[:, b, :], OB[:, v0:v0 + T], ACTF.Relu,
                            bias=Vt[:, VCPF:VCPF + 1],
                            scale=Vt[:, VKPF:VKPF + 1],
                            accum_out=Mt[:, col:col + 1])
                    return lambda: nc.vector.scalar_tensor_tensor(
                        scrp[:, b, :], OB[:, v0:v0 + T],
                        Vt[:, VKPF:VKPF + 1],
                        Vt[:, VNCPF:VNCPF + 1].to_broadcast([128, T]),
                        ALU.mult, ALU.max, accum_out=Mt[:, col:col + 1])

                def mk_rb(h, nxt=nxt, Vt=Vt):
                    rbv = Rb[:, lo:hi].rearrange("p (b q) -> p b q", q=P)
                    obv = OB[:, lo:hi].rearrange("p (b q) -> p b q", q=P)
                    nxv = nxt[:, lo:hi].rearrange("p (b q) -> p b q", q=P)
                    sl = slice(2 * h, 2 * h + 2)
                    return lambda: nc.vector.tensor_tensor(
                        rbv[:, sl, 2:T + 2], obv[:, sl, 2:T + 2],
                        nxv[:, sl, 2:T + 2], ALU.add)

                deferred = [mk_pool(0), mk_pool(1)]
                if i < NOUT - 1:
                    deferred += [mk_rb(0), mk_rb(1)]
                deferred += [mk_pool(2), mk_pool(3)]
                if i == NOUT - 1:
                    while deferred:
                        deferred.pop(0)()
                spx = nxt
                if nW is not None:
                    LWt, LWf, Vt = nW

            # final-pool contribution of raw spx[7]: relu(kf*x + cf)
            sxv = spx[:, lo:hi].rearrange("p (b q) -> p b q", q=P)
            for b in range(BL):
                col = BL * 7 + b
                if b < 2:
                    nc.scalar.activation(scrp[:, b, :], sxv[:, b, 2:T + 2],
                                         ACTF.Relu, bias=vec7_s[:, 1:2],
                                         scale=vec7_s[:, 0:1],
                                         accum_out=Mt[:, col:col + 1])
                else:
                    nc.vector.scalar_tensor_tensor(
                        scrp[:, b, :], sxv[:, b, 2:T + 2], vec7_s[:, 0:1],
                        vec7_s[:, 2:3].to_broadcast([128, T]),
                        ALU.mult, ALU.max, accum_out=Mt[:, col:col + 1])

            # classifier: y = sum_g fcw_g.T @ M_g + fcb
            nc.vector.tensor_tensor(Mt_r[:], Mt[:], tcpf_s[:], ALU.add)
            fps = spsum.tile([2, BL], F32, tag="sps", name="fps")
            for g in range(8):
                nc.tensor.matmul(fps, fcw_s[:, 2 * g:2 * g + 2],
                                 Mt_r[:, BL * g:BL * (g + 1)],
                                 start=(g == 0), stop=(g == 7))
            nc.scalar.activation(outs[:], fps, ACTF.Identity,
                                 bias=fcb_s[:], scale=1.0)
            nc.sync.dma_start(out=y_d[:], in_=outs[:])

    return nc


_NC_CACHE = {}


def _get_nc(T):
    if T not in _NC_CACHE:
        nc = build(T)
        _split_waits(nc)
        _NC_CACHE[T] = nc
    return _NC_CACHE[T]


def make_in_maps(inputs):
    x = np.asarray(inputs["x"], np.float32)
    T = x.shape[2]
    lw8, lwf, vec, tcpf, vec7, fcw, fcb = _prep(inputs, T)
    x8 = np.ascontiguousarray(x.astype(E4))
    xr = np.ascontiguousarray(x[:, 0:128, :].astype(np.float16))
    in_maps = []
    for core in range(NCORES):
        in_maps.append({
            "x": np.ascontiguousarray(x8[core * BL:(core + 1) * BL]),
            "xr": np.ascontiguousarray(xr[core * BL:(core + 1) * BL]),
            "lw8": lw8, "lwf": lwf, "vec": vec, "tcpf": tcpf,
            "vec7": vec7, "fcw": fcw, "fcb": fcb,
        })
    return in_maps, T


def kernel(**inputs):
    in_maps, T = make_in_maps(inputs)
    nc = _get_nc(T)
    res = run_bass_kernel_spmd(nc, in_maps, list(range(NCORES)))
    out = np.concatenate(
        [np.asarray(res.results[c]["y"]).T for c in range(NCORES)], axis=0)
    return np.ascontiguousarray(out.astype(np.float32))


# revision 17
# speedup vs baseline: 1.2467x; 1.0039x over previous
"""Trainium2 Bass kernel for nn_Nested_Res2Net_TDNN (B=32, CIN=1024, T=600).

Sharding: data-parallel over batch across 8 NeuronCores (4 per core),
parameters replicated.

v2: fp8(e4m3) DoubleRow matmuls.  Data stays channels-on-128-partitions,
(batch x time + guard pads) on the free dim, exactly as the fp16 version;
every conv becomes fp8 DoubleRow pairs at 0.5 cycles/column:
  - the 3 dilated taps of each scale branch pair as (tap-2, tap0) via a
    stride-2 rhs access pattern, plus (tap+2, fp8-error-compensation) via
    a stride-0 pair,
  - the 1x1 convs (conv1 spx/OB parts, conv3 terms) pair each fp8 weight
    plane with its own fp8 quantization-error plane (stride-0 rhs), which
    recovers most of the weight-quantization error for free,
  - the conv3 terms pair with each other via slab-alternating branch
    state (z_j written to slab (j+1)%2), an SBUF->SBUF DMA refreshes the
    few pristine rows the odd slab needs,
  - BN affines fold into following-matmul weights (rows scaled by k) with
    -c/k guard-pad values so dilated taps see exact zero padding; pad
    values are rewritten per PSUM chunk (not per branch) so the next
    branch's taps pipeline chunk-by-chunk instead of serializing,
  - post-matmul relu/bias writes alternate ACT/DVE (GPSIMD cannot read
    PSUM); GPSIMD gets pad writes and the residual add,
  - the SE/tail fuses via scalar_tensor_tensor: u=(z3*k3g)+res in place,
    OB'=relu(ko*u + ko*(gate*c3+co_prev)) (pre-scaled by obn k so the
    residual build is a plain add), pooling accum rides ACT/DVE,
    next-block conv1-OB matmuls are staggered into the tail per batch,
    and off-critical pool/residual ops are deferred into the next
    block's branch slots to keep them behind critical queue work.
fp16 is kept for the SE squeeze path, O3/residual tensors and the
classifier; PSUM is fp32 throughout.
"""

import sys

for _p in ("/opt/trn_rl_repo",):
    if _p not in sys.path:
        sys.path.insert(0, _p)

import numpy as np
import ml_dtypes

import concourse.bass as bass
import concourse.mybir as mybir
import concourse.tile as tile
from concourse.bass_utils import run_bass_kernel_spmd
from bass_rust import AP as RAP

B, CIN, T0 = 32, 1024, 600
NES, SCALE = 8, 8
CBLK, WSC = 128, 16
NIN, NOUT = 7, 7
EPS = 1e-5
NCORES = 8
BL = B // NCORES

F32 = mybir.dt.float32
F16 = mybir.dt.float16
F8 = mybir.dt.float8e4
E4 = ml_dtypes.float8_e4m3
ALU = mybir.AluOpType
ACTF = mybir.ActivationFunctionType
DR = mybir.MatmulPerfMode.DoubleRow

# DoubleRow weight slots (each [K=128, 2 planes, 128 out]) per block
SC1A = 0                      # conv1 spx part: (W1p, err)
SC1B = 1                      # conv1 OB part: (W1B, err)
STAPA = lambda j: 2 + 2 * j   # branch j taps (-2, 0)
STAPB = lambda j: 3 + 2 * j   # branch j (tap +2, err)
SC3P = lambda m: 16 + m       # conv3 slab pairs: (IX7,c0),(c1,c2),(c3,c4),(c5,c6)
NDR = 20

# VEC columns (fp32 per-partition scalars)
VB1, VPAD1 = 0, 1
VBRB = lambda j: 2 + 2 * j    # branch post bias
VBRP = lambda j: 3 + 2 * j    # branch pad value -c/k
VB3, VKO, VK3, VC3, VCOP = 16, 17, 18, 19, 20
VKPF, VCPF, VSE1B, VSE2B, VNCPF = 21, 22, 23, 24, 25
NV = 26


def _perm():
    """ZS slice order: s=0 is spx[7] (channels 112:128), s>=1 is spx[s-1]."""
    p = np.zeros(128, np.int64)
    for s in range(8):
        for c in range(16):
            p[16 * s + c] = 112 + c if s == 0 else 16 * (s - 1) + c
    return p


def _bnkc(p):
    g, b, m, v = [np.asarray(a, np.float64) for a in p]
    k = g / np.sqrt(v + EPS)
    return k, b - m * k


def _q8(a):
    return np.asarray(a, np.float32).astype(E4).astype(np.float64)


def _prep(inp, T):
    f = lambda n: np.asarray(inp[n], np.float64)
    w1, b1, bn1 = f("w1"), f("b1"), f("bn1")
    cw, cb, ibn = f("cw"), f("cb"), f("ibn")
    w3, b3, bn3 = f("w3"), f("b3"), f("bn3")
    se1w, se1b = f("se1w"), f("se1b")
    se2w, se2b = f("se2w"), f("se2b")
    obn, fbn = f("obn"), f("fbn")
    fcw, fcb = f("fcw"), f("fcb")
    ws = [f(f"ws{j}") for j in range(NIN)]
    perm = _perm()
    kf, cf = _bnkc(fbn)  # (1024,)

    LW8 = np.zeros((NOUT, 128, NDR, 2, 128), np.float64)
    LWF = np.zeros((NOUT, 128, 2, 128), np.float64)
    VEC = np.zeros((NOUT, 128, NV), np.float64)
    TCPF = np.zeros((128, NES * BL), np.float64)

    def put_pair(i, slot, Wm):
        """plane0 = fp8(W), plane1 = fp8(W - plane0)."""
        p0 = _q8(Wm)
        LW8[i, :, slot, 0] = p0
        LW8[i, :, slot, 1] = _q8(Wm - p0)

    for i in range(NOUT):
        k1, c1 = _bnkc(bn1[i])
        k1p, c1p = k1[perm], c1[perm]
        kb, cbv = [], []
        for j in range(NIN):
            kj, cj = _bnkc(ibn[i, j])
            kb.append(kj)
            cbv.append(cj)
        assert np.abs(k1).min() > 1e-3
        assert min(np.abs(k).min() for k in kb) > 1e-3

        def row_kc(j, k1p=k1p, c1p=c1p, kb=kb, cbv=cbv):
            k = k1p.copy()
            c = c1p.copy()
            if j > 0:
                n = 16 * (j + 1)
                k[:n] = np.tile(kb[j - 1], j + 1)
                c[:n] = np.tile(cbv[j - 1], j + 1)
            return k, c

        put_pair(i, SC1A, w1[i][perm, :].T)
        ko, co = _bnkc(obn[i])
        if i > 0:
            # OB is stored pre-scaled by ko_prev, so no fold here
            put_pair(i, SC1B, w1[i][perm, :].T)

        for j in range(NIN):
            krow, crow = row_kc(j)
            blk = [cw[i, j, :, :, d, 0].T for d in range(3)]  # [ci, co]
            tapm = [np.zeros((128, 128)) for _ in range(3)]
            bias_full = np.zeros(128)
            for s in range(j + 2):
                r0 = 16 * s
                const_s = np.zeros(16)
                for d in range(3):
                    tapm[d][r0:r0 + 16, r0:r0 + 16] = \
                        blk[d] * krow[r0:r0 + 16, None]
                    const_s += blk[d].T @ crow[r0:r0 + 16]
                bias_full[r0:r0 + 16] = cb[i, j] + const_s
            # slot A: (tap-2, tap0); slot B: (tap+2, err(tap+2))
            LW8[i, :, STAPA(j), 0] = _q8(tapm[0])
            LW8[i, :, STAPA(j), 1] = _q8(tapm[1])
            put_pair(i, STAPB(j), tapm[2])
            VEC[i, :, VBRB(j)] = bias_full
            VEC[i, :16 * (j + 2), VBRP(j)] = \
                np.tile(-cbv[j] / kb[j], j + 2)

        # conv3 terms, packed as slab pairs (no err compensation)
        c3const = np.zeros(128)
        IX7 = np.zeros((128, 128))
        IX7[0:16, :] = w3[i][:, 112:128].T * k1p[0:16, None]
        c3const += w3[i][:, 112:128] @ c1p[0:16]
        IC3s = []
        for j in range(NIN):
            wj = ws[j][i]
            IC3 = np.zeros((128, 128))
            for s in range(j + 2):
                r0 = 16 * s
                IC3[r0:r0 + 16, :] = \
                    (wj[s] * w3[i][:, 16 * j:16 * j + 16].T) * kb[j][:, None]
                c3const += wj[s] * (w3[i][:, 16 * j:16 * j + 16] @ cbv[j])
            IC3s.append(IC3)
        # pair m reads (slab0-term, slab1-term) at the post(2m) epoch
        pairs3 = [(IX7, IC3s[0]), (IC3s[1], IC3s[2]),
                  (IC3s[3], IC3s[4]), (IC3s[5], IC3s[6])]
        for m, (pa, pb) in enumerate(pairs3):
            LW8[i, :, SC3P(m), 0] = _q8(pa)
            LW8[i, :, SC3P(m), 1] = _q8(pb)

        k3, c3 = _bnkc(bn3[i])
        LWF[i, :, 0, 0:16] = (se1w[i] * k3[None, :] / T).T
        LWF[i, 0:16, 1, :] = se2w[i].T

        VEC[i, :, VB1] = b1[i][perm]
        if i > 0:
            _, co_p = _bnkc(obn[i - 1])
            VEC[i, :, VB1] += (w1[i] @ co_p)[perm]
            VEC[i, :, VCOP] = ko * co_p   # obias' carries the ko prescale
        VEC[i, :, VPAD1] = -c1p / k1p
        VEC[i, :, VB3] = b3[i] + c3const
        VEC[i, :, VKO] = ko
        VEC[i, :, VK3], VEC[i, :, VC3] = k3, ko * c3
        kfi = kf[128 * i:128 * (i + 1)]
        cfi = cf[128 * i:128 * (i + 1)]
        VEC[i, :, VKPF] = kfi           # OB' = ko*OB already
        VEC[i, :, VCPF] = kfi * co + cfi
        VEC[i, :, VNCPF] = -(kfi * co + cfi)
        for bb in (1, 3):  # only max-form (DVE) pool columns need +T*cpf
            TCPF[:, BL * i + bb] = T * (kfi * co + cfi)
        VEC[i, :16, VSE1B] = se1b[i] + se1w[i] @ c3
        VEC[i, :, VSE2B] = se2b[i]
    TCPF[:, 30:32] = (T * cf[896:1024])[:, None]  # x7 b2,b3 use max-form
    vec7 = np.stack([kf[896:1024], cf[896:1024], -cf[896:1024]], axis=1)
    fcwp = np.zeros((128, 8, 2))
    for g in range(8):
        fcwp[:, g, :] = (fcw[:, 128 * g:128 * (g + 1)] / T).T
    lw8 = np.ascontiguousarray(
        LW8.reshape(NOUT, 128, NDR * 256).astype(E4))
    lwf = np.ascontiguousarray(
        LWF.reshape(NOUT, 128, 256).astype(np.float16))
    vec = np.ascontiguousarray(VEC.astype(np.float32))
    return (lw8, lwf, vec,
            np.ascontiguousarray(TCPF.astype(np.float32)),
            np.ascontiguousarray(vec7.astype(np.float32)),
            np.ascontiguousarray(fcwp.reshape(128, 16).astype(np.float16)),
            np.ascontiguousarray(fcb.reshape(2, 1).astype(np.float32)))


def _split_waits(nc, max_waits=1):
    """walrus's TRN2 codegen rejects >1 sync wait on drain/matmul (and
    possibly other) instructions; peel extras onto preceding single-wait
    no-ops on the same engine."""
    n_new = 0
    for fn in nc.m.functions:
        for bb in fn.blocks:
            out = []
            for ins in bb.instructions:
                si = ins.sync_info
                if si is not None and len(si.on_wait) > max_waits:
                    waits = list(si.on_wait)
                    for w in waits[max_waits:]:
                        nop = mybir.InstNoOp(
                            name=f"I-splitwait-{n_new}",
                            sync_info=mybir.SyncInfo(on_wait=[w], on_update=[]),
                            bass_nofuse=True,
                            engine=ins.engine,
                        )
                        out.append(nop)
                        n_new += 1
                    ins.sync_info = mybir.SyncInfo(
                        on_wait=waits[:max_waits], on_update=list(si.on_update))
                out.append(ins)
            bb.instructions = out
    return n_new


def build(T=T0):
    nc = bass.Bass("TRN2")
    P = T + 4
    W2 = BL * P + 4
    lo, hi = 2, 2 + BL * P
    chunks = []
    c = lo
    while c < hi:
        w = min(512, hi - c)
        chunks.append((c, w))
        c += w
    NCH = len(chunks)
    # valid-column pieces (for bn3 squeeze accumulation)
    pieces = []
    for b in range(BL):
        v0, v1 = 4 + P * b, 4 + P * b + T
        for ci, (c0, w) in enumerate(chunks):
            s, e = max(v0, c0), min(v1, c0 + w)
            if s < e:
                pieces.append((ci, s, e - s, b))
    NP = len(pieces)
    assert NP == 2 * BL and all(p[3] == k // 2 for k, p in enumerate(pieces))

    x_d = nc.dram_tensor("x", [BL, CIN, T], F8, kind="ExternalInput")
    xr_d = nc.dram_tensor("xr", [BL, 128, T], F16, kind="ExternalInput")
    lw8_d = nc.dram_tensor("lw8", [NOUT, 128, NDR * 256], F8,
                           kind="ExternalInput")
    lwf_d = nc.dram_tensor("lwf", [NOUT, 128, 256], F16, kind="ExternalInput")
    vec_d = nc.dram_tensor("vec", [NOUT, 128, NV], F32, kind="ExternalInput")
    tcpf_d = nc.dram_tensor("tcpf", [128, NES * BL], F32, kind="ExternalInput")
    vec7_d = nc.dram_tensor("vec7", [128, 3], F32, kind="ExternalInput")
    fcw_d = nc.dram_tensor("fcw", [128, 16], F16, kind="ExternalInput")
    fcb_d = nc.dram_tensor("fcb", [2, 1], F32, kind="ExternalInput")
    y_d = nc.dram_tensor("y", [2, BL], F32, kind="ExternalOutput")

    def pads_ap(t, nact, slab=None):
        """All guard/pad columns: 4-col blocks at 0, P, 2P, 3P, 4P."""
        a = t[:]
        off = a.offset + (0 if slab is None else slab * W2)
        return RAP(a.tensor, off,
                   [[int(a.ap[0][0]), nact], [P, BL + 1], [1, 4]])

    def pair(t, base, stride, w, np_=128, slab=None):
        a = t[:]
        off = a.offset + base + (0 if slab is None else slab * W2)
        return RAP(a.tensor, off,
                   [[int(a.ap[0][0]), np_], [stride, 2], [1, w]])

    with tile.TileContext(nc) as tc:
        with tc.tile_pool(name="state", bufs=1) as state, \
             tc.tile_pool(name="spxp", bufs=2) as spxp, \
             tc.tile_pool(name="wp", bufs=2) as wp, \
             tc.tile_pool(name="vp", bufs=2) as vp, \
             tc.tile_pool(name="bpsum", bufs=2, space="PSUM") as bpsum, \
             tc.tile_pool(name="cpsum", bufs=NCH, space="PSUM") as cpsum, \
             tc.tile_pool(name="spsum", bufs=1, space="PSUM") as spsum:

            ZS2 = state.tile([128, 2, W2], F8, tag="ZS2")
            OB = state.tile([128, W2], F8, tag="OB")
            Rb = state.tile([128, W2], F16, tag="Rb")
            O3 = state.tile([128, W2], F16, tag="O3")
            scrp = state.tile([128, BL, T], F16, tag="scrp")
            sq8 = state.tile([128, NP], F32, tag="sq8")
            sq_r = state.tile([128, BL], F16, tag="sq_r")
            seh = state.tile([128, BL], F16, tag="seh")
            gate = state.tile([128, BL], F32, tag="gate")
            k3g = state.tile([128, BL], F32, tag="k3g")
            obias = state.tile([128, BL], F32, tag="obias")
            Mt = state.tile([128, NES * BL], F32, tag="Mt")
            Mt_r = state.tile([128, NES * BL], F16, tag="Mt_r")
            zero16 = state.tile([128, 1], F16, tag="z16")
            outs = state.tile([2, BL], F32, tag="outs")
            fcw_s = state.tile([128, 16], F16, tag="fcw")
            fcb_s = state.tile([2, 1], F32, tag="fcb")
            tcpf_s = state.tile([128, NES * BL], F32, tag="tcpf")
            vec7_s = state.tile([128, 3], F32, tag="vec7")

            # conv3 slab pairs read all 128 slab1 rows (zero weights on
            # unused rows) before the first block fully populates them
            nc.gpsimd.memset(ZS2[32:64, 1, :], 0)
            nc.gpsimd.memset(ZS2[64:128, 1, :], 0)
            nc.gpsimd.memset(OB[:], 0)
            nc.vector.memset(zero16[:], 0)

            def load_spx(i):
                t = spxp.tile([128, W2], F8, tag="spx")
                nc.gpsimd.memset(pads_ap(t, 128), 0)
                nc.sync.dma_start(
                    out=t[:, lo:hi].rearrange("p (b q) -> p b q", q=P)[:, :, 2:T + 2],
                    in_=x_d[:, 128 * i:128 * (i + 1), :].rearrange("b c t -> c b t"))
                return t

            def load_w(i):
                t = wp.tile([128, NDR, 256], F8, tag="lw8")
                nc.sync.dma_start(
                    out=t[:], in_=lw8_d[i].rearrange("k (s m) -> k s m", m=256))
                tf = wp.tile([128, 2, 128], F16, tag="lwf")
                nc.sync.dma_start(
                    out=tf[:], in_=lwf_d[i].rearrange("k (s m) -> k s m", m=128))
                v = vp.tile([128, NV], F32, tag="vec")
                nc.sync.dma_start(out=v[:], in_=vec_d[i])
                return t, tf, v

            # post-op engine rotation.  GPSIMD cannot read PSUM on HW, so
            # PSUM drains go on ACT/DVE only (ACT is faster per column).
            _rr = [0]
            _pat = "aadadadadadad"  # 8 ACT : 5 DVE

            def post_chunk(dst, ps, nact, biasap, accum=None):
                e = _pat[_rr[0] % len(_pat)]
                _rr[0] += 1
                if e == "a":
                    nc.scalar.activation(dst, ps, ACTF.Relu, bias=biasap,
                                         scale=1.0, accum_out=accum)
                else:
                    if accum is None:
                        nc.vector.tensor_scalar(dst, ps, biasap, 0.0,
                                                ALU.add, ALU.max)
                    else:
                        nc.vector.scalar_tensor_tensor(
                            dst, ps, biasap,
                            zero16[:nact, 0:1].to_broadcast(list(dst.shape)),
                            ALU.add, ALU.max, accum_out=accum)

            def pad_blk(t, nact, Vt, col, blkidx, slab):
                a = t[:]
                off = a.offset + slab * W2 + P * blkidx
                ap = RAP(a.tensor, off, [[int(a.ap[0][0]), nact], [1, 4]])
                bc = Vt[:nact, col:col + 1].to_broadcast([nact, 4])
                nc.gpsimd.tensor_copy(out=ap, in_=bc)

            def post_pass(dst, psl, nact, bias_col, Vt, padt=None,
                          pad_col=None, pad_slab=0):
                biasap = Vt[:nact, bias_col:bias_col + 1]
                for k, (c0, w) in enumerate(chunks):
                    post_chunk(dst[:nact, c0:c0 + w], psl[k][:nact, :w],
                               nact, biasap)
                    if padt is not None:
                        pad_blk(padt, nact, Vt, pad_col, k, pad_slab)

            def w8ap(Wt, slot, nact=128):
                return Wt[:, slot, :].rearrange("k (a m) -> k a m", a=2)[:, :, 0:nact]

            spx = load_spx(0)
            LWt, LWf, Vt = load_w(0)
            # bulky/late-needed loads go after block-0's critical DMAs
            nc.sync.dma_start(
                out=Rb[:, lo:hi].rearrange("p (b q) -> p b q", q=P)[:, :, 2:T + 2],
                in_=xr_d.rearrange("b c t -> c b t"))
            nc.sync.dma_start(out=fcw_s[:], in_=fcw_d[:])
            nc.sync.dma_start(out=fcb_s[:], in_=fcb_d[:])
            nc.sync.dma_start(out=vec7_s[:], in_=vec7_d[:])
            nc.sync.dma_start(out=tcpf_s[:], in_=tcpf_d[:])
            pend_c1 = None
            deferred = []  # off-critical ops from the previous block's tail

            for i in range(NOUT):
                nxt = load_spx(i + 1)
                nW = load_w(i + 1) if i < NOUT - 1 else None

                # conv1 psum: spx part was issued in block i-1's tail
                if i == 0:
                    c1ps = [cpsum.tile([128, 512], F32, tag="cps",
                                       name="c1ps")[:, :w] for (c0, w) in chunks]
                    for k, (c0, w) in enumerate(chunks):
                        nc.tensor.matmul(c1ps[k], w8ap(LWt, SC1A),
                                         pair(spx, c0, 0, w),
                                         start=True, stop=True, perf_mode=DR)
                else:
                    c1ps = pend_c1  # completed (SC1B) in prev block's tail
                zs0 = ZS2[:, 0, :]
                post_pass(zs0, c1ps, 128, VB1, Vt,
                          padt=ZS2, pad_col=VPAD1, pad_slab=0)
                # pristine rows needed by odd branches on slab 1 (incl pads)
                for r0 in (32, 64, 96):
                    nc.sync.dma_start(out=ZS2[r0:r0 + 16, 1, :],
                                      in_=ZS2[r0:r0 + 16, 0, :])

                # conv3 accumulation group (filled by slab pairs below)
                cps = [cpsum.tile([128, 512], F32, tag="cps", name="cps")[:, :w]
                       for (c0, w) in chunks]

                # inner scale branches, in place in ZS
                for j in range(NIN):
                    nact = 16 * (j + 2)
                    rj, wj = j % 2, (j + 1) % 2
                    bps = [bpsum.tile([128, 512], F32, tag="bps",
                                      name="bps")[:, :w] for (c0, w) in chunks]
                    # chunk-pair interleave: with bufs=2 PSUM banks, group k
                    # must close (TAPB) before bank reuse at chunk k+2
                    for kk in range(0, NCH, 2):
                        ks = [k for k in (kk, kk + 1) if k < NCH]
                        for k in ks:
                            c0, w = chunks[k]
                            nc.tensor.matmul(bps[k][:nact, :],
                                             w8ap(LWt, STAPA(j), nact),
                                             pair(ZS2, c0 - 2, 2, w, slab=rj),
                                             start=True, stop=False,
                                             perf_mode=DR)
                        for k in ks:
                            c0, w = chunks[k]
                            nc.tensor.matmul(bps[k][:nact, :],
                                             w8ap(LWt, STAPB(j), nact),
                                             pair(ZS2, c0 + 2, 0, w, slab=rj),
                                             start=False, stop=True,
                                             perf_mode=DR)
                    post_pass(ZS2[:, wj, :], bps, nact, VBRB(j), Vt,
                              padt=ZS2, pad_col=VBRP(j), pad_slab=wj)
                    if deferred:
                        deferred.pop(0)()
                    if j % 2 == 0:
                        # conv3 slab pair m=j//2: (slab0 term, slab1 term)
                        for k, (c0, w) in enumerate(chunks):
                            nc.tensor.matmul(cps[k], w8ap(LWt, SC3P(j // 2)),
                                             pair(ZS2, c0, W2, w),
                                             start=(j == 0), stop=(j == 6),
                                             perf_mode=DR)

                while deferred:
                    deferred.pop(0)()
                # bn3 relu -> O3 (fp16), per-piece accum for SE squeeze
                for kpc, (ci, s, w, b) in enumerate(pieces):
                    c0 = chunks[ci][0]
                    post_chunk(O3[:, s:s + w], cps[ci][:, s - c0:s - c0 + w],
                               128, Vt[:, VB3:VB3 + 1],
                               accum=sq8[:, kpc:kpc + 1])
                # next block's conv1 spx-part (fills the SE gap)
                if nW is not None:
                    pend_c1 = [cpsum.tile([128, 512], F32, tag="cps",
                                          name="c1ps")[:, :w]
                               for (c0, w) in chunks]
                    for k, (c0, w) in enumerate(chunks):
                        nc.tensor.matmul(pend_c1[k], w8ap(nW[0], SC1A),
                                         pair(nxt, c0, 0, w),
                                         start=True, stop=False, perf_mode=DR)

                # SE squeeze/excite (fp16)
                nc.vector.tensor_tensor(
                    sq_r[:], sq8[:, 0:NP:2], sq8[:, 1:NP:2], ALU.add)
                ps1 = spsum.tile([128, BL], F32, tag="sps")
                nc.tensor.matmul(ps1, LWf[:, 0, :], sq_r[:],
                                 start=True, stop=True)
                nc.scalar.activation(seh[:], ps1, ACTF.Relu,
                                     bias=Vt[:, VSE1B:VSE1B + 1], scale=1.0)
                ps2 = spsum.tile([128, BL], F32, tag="sps")
                nc.tensor.matmul(ps2, LWf[:, 1, :], seh[:],
                                 start=True, stop=True)
                nc.scalar.activation(gate[:], ps2, ACTF.Sigmoid,
                                     bias=Vt[:, VSE2B:VSE2B + 1], scale=1.0)
                # gate-scaled scalars: k3g = gate*k3, obias = gate*c3 + co_prev
                nc.vector.tensor_tensor(
                    k3g[:], gate[:],
                    Vt[:, VK3:VK3 + 1].to_broadcast([128, BL]), ALU.mult)
                nc.vector.scalar_tensor_tensor(
                    obias[:], gate[:], Vt[:, VC3:VC3 + 1],
                    Vt[:, VCOP:VCOP + 1].to_broadcast([128, BL]),
                    ALU.mult, ALU.add)

                # tail per batch: u = O3*k3g + res (in place);
                # OB = relu(u + obias); pool accum; Rb = OB*ko + nxt
                c1b_chunks = {0: [0], 1: [1], 2: [2], 3: [3, 4]}
                for b in range(BL):
                    v0 = 4 + P * b
                    o3b = O3[:, v0:v0 + T]
                    nc.vector.scalar_tensor_tensor(
                        o3b, o3b, k3g[:, b:b + 1], Rb[:, v0:v0 + T],
                        ALU.mult, ALU.add)
                    obb = OB[:, v0:v0 + T]
                    nc.scalar.activation(obb, o3b, ACTF.Relu,
                                         bias=obias[:, b:b + 1],
                                         scale=Vt[:, VKO:VKO + 1])
                    # complete next block's conv1 psum group on the chunks
                    # this batch unblocks (PE gets work while tail runs)
                    if nW is not None:
                        for k in c1b_chunks[b]:
                            c0, w = chunks[k]
                            nc.tensor.matmul(pend_c1[k], w8ap(nW[0], SC1B),
                                             pair(OB, c0, 0, w),
                                             start=False, stop=True,
                                             perf_mode=DR)
                # pooling + residual build are off the critical path:
                # defer them into the next block's branch slots so they sit
                # behind the critical posts in the engine queues
                def mk_pool(b, i=i, Vt=Vt):
                    v0 = 4 + P * b
                    col = BL * i + b
                    if b % 2 == 0:
                        return lambda: nc.scalar.activation(
                            scrp[:, b, :], OB[:, v0:v0 + T], ACTF.Relu,
                            bias=Vt[:, VCPF:VCPF + 1],
                            scale=Vt[:, VKPF:VKPF + 1],
                            accum_out=Mt[:, col:col + 1])
                    return lambda: nc.vector.scalar_tensor_tensor(
                        scrp[:, b, :], OB[:, v0:v0 + T],
                        Vt[:, VKPF:VKPF + 1],
                        Vt[:, VNCPF:VNCPF + 1].to_broadcast([128, T]),
                        ALU.mult, ALU.max, accum_out=Mt[:, col:col + 1])

                def mk_rb(h, nxt=nxt, Vt=Vt):
                    rbv = Rb[:, lo:hi].rearrange("p (b q) -> p b q", q=P)
                    obv = OB[:, lo:hi].rearrange("p (b q) -> p b q", q=P)
                    nxv = nxt[:, lo:hi].rearrange("p (b q) -> p b q", q=P)
                    sl = slice(2 * h, 2 * h + 2)
                    return lambda: nc.vector.tensor_tensor(
                        rbv[:, sl, 2:T + 2], obv[:, sl, 2:T + 2],
                        nxv[:, sl, 2:T + 2], ALU.add)

                deferred = [mk_pool(0), mk_pool(1)]
                if i < NOUT - 1:
                    deferred += [mk_rb(0), mk_rb(1)]
                deferred += [mk_pool(2), mk_pool(3)]
                if i == NOUT - 1:
                    while deferred:
                        deferred.pop(0)()
                spx = nxt
                if nW is not None:
                    LWt, LWf, Vt = nW

            # final-pool contribution of raw spx[7]: relu(kf*x + cf)
            sxv = spx[:, lo:hi].rearrange("p (b q) -> p b q", q=P)
            for b in range(BL):
                col = BL * 7 + b
                if b < 2:
                    nc.scalar.activation(scrp[:, b, :], sxv[:, b, 2:T + 2],
                                         ACTF.Relu, bias=vec7_s[:, 1:2],
                                         scale=vec7_s[:, 0:1],
                                         accum_out=Mt[:, col:col + 1])
                else:
                    nc.vector.scalar_tensor_tensor(
                        scrp[:, b, :], sxv[:, b, 2:T + 2], vec7_s[:, 0:1],
                        vec7_s[:, 2:3].to_broadcast([128, T]),
                        ALU.mult, ALU.max, accum_out=Mt[:, col:col + 1])

            # classifier: y = sum_g fcw_g.T @ M_g + fcb
            nc.vector.tensor_tensor(Mt_r[:], Mt[:], tcpf_s[:], ALU.add)
            fps = spsum.tile([2, BL], F32, tag="sps", name="fps")
            for g in range(8):
                nc.tensor.matmul(fps, fcw_s[:, 2 * g:2 * g + 2],
                                 Mt_r[:, BL * g:BL * (g + 1)],
                                 start=(g == 0), stop=(g == 7))
            nc.scalar.activation(outs[:], fps, ACTF.Identity,
                                 bias=fcb_s[:], scale=1.0)
            nc.sync.dma_start(out=y_d[:], in_=outs[:])

    return nc


_NC_CACHE = {}


def _get_nc(T):
    if T not in _NC_CACHE:
        nc = build(T)
        _split_waits(nc)
        _NC_CACHE[T] = nc
    return _NC_CACHE[T]


def make_in_maps(inputs):
    x = np.asarray(inputs["x"], np.float32)
    T = x.shape[2]
    lw8, lwf, vec, tcpf, vec7, fcw, fcb = _prep(inputs, T)
    x8 = np.ascontiguousarray(x.astype(E4))
    xr = np.ascontiguousarray(x[:, 0:128, :].astype(np.float16))
    in_maps = []
    for core in range(NCORES):
        in_maps.append({
            "x": np.ascontiguousarray(x8[core * BL:(core + 1) * BL]),
            "xr": np.ascontiguousarray(xr[core * BL:(core + 1) * BL]),
            "lw8": lw8, "lwf": lwf, "vec": vec, "tcpf": tcpf,
            "vec7": vec7, "fcw": fcw, "fcb": fcb,
        })
    return in_maps, T


def kernel(**inputs):
    in_maps, T = make_in_maps(inputs)
    nc = _get_nc(T)
    res = run_bass_kernel_spmd(nc, in_maps, list(range(NCORES)))
    out = np.concatenate(
        [np.asarray(res.results[c]["y"]).T for c in range(NCORES)], axis=0)
    return np.ascontiguousarray(out.astype(np.float32))
